# revision 26
# baseline (speedup 1.0000x reference)
"""Trainium2 Bass kernel for an attention-augmented LSTM (CaptioningRNN).

Reference computation (per batch n, T timesteps):
    A_flat = A.reshape(N, H, 16); h0 = c0 = A_flat.mean(-1)
    scores_t = (h_{t-1} @ A_flat) / sqrt(H); w = softmax(scores)
    attn_t = A_flat @ w
    a = x_t @ Wx + h_{t-1} @ Wh + attn_t @ Wattn + b
    i, f, o, g = split(a, 4); c_t = sig(f)*c + sig(i)*tanh(g); h_t = sig(o)*tanh(c_t)

Strategy: data-parallel over batch across 8 cores (32 batch rows each).
Per core:
  Phase A: U = x @ Wx + b, streamed in row-group-blocked units (Wx read 4x
           instead of 16x), staged to DRAM bf16, paced 2 units/step into the
           recurrence's idle windows.
  Phase B: 64 recurrent steps. Gate matmul = [h; attn] (2048-dim contraction,
           bf16) against W2 = [Wh; Wattn] with gate-interleaved columns so each
           512-column block yields a full 128-dim slice of (i,f,o,g) and thus a
           128-dim slice of h/c. Attention scores on the tensor engine (all
           batch pairs, diagonal via mask + strided reduce). Attention POOLING
           also on the tensor engine: softmax weights are scattered into a
           block-diagonal [128np, 4x32n] operand (stream-transpose + replicated
           DMA + static mask) and contracted against a (n,p)-major copy of A
           (atp), yielding attnT chunks directly in [h,n] layout. h transposed
           back to hT layout with PE transpose-mode matmuls (not DMA).

Weight-matrix column order (gate interleave): block j (512 cols) holds
original columns [i|f|o|g][j*128:(j+1)*128]. The same permutation is applied
to Wx, b and hence U.
"""

import math
import os

import numpy as np
import ml_dtypes

import concourse.bass as bass
import concourse.mybir as mybir
import concourse.tile as tile
from concourse import bacc

N, T, D, H = 256, 64, 1024, 1024
NCORES = 8
NB = N // NCORES          # 32 batch rows per core
G = 4 * H                 # 4096 gate columns
P = 16                    # attention positions (4x4)
KH = H // 128             # 8 contraction chunks for h
K2 = (2 * H) // 128       # 16 contraction chunks for [h; attn]
GB = G // 512             # 8 gate blocks of 512
NPC = (NB * P) // 128     # 4 chunks of the (n,p) axis
F32 = mybir.dt.float32
BF16 = mybir.dt.bfloat16
BF = ml_dtypes.bfloat16

AF = mybir.ActivationFunctionType
ALU = mybir.AluOpType
AXX = mybir.AxisListType.X

_NC_CACHE = {}

# phase A row-group blocking: 4 groups x 4 row-tiles
MG = 4                    # row-tile groups
MPG = 4                   # row-tiles per group (each row-tile = 4 timesteps)
N_ROW_TILES = MG * MPG    # 16


def _gate_perm():
    """perm[new_col] = old_col for the gate-interleaved layout."""
    perm = np.empty(G, dtype=np.int64)
    for j in range(GB):
        for s in range(4):  # i, f, o, g
            perm[j * 512 + s * 128:(j * 512 + (s + 1) * 128)] = np.arange(
                s * H + j * 128, s * H + (j + 1) * 128)
    return perm


def build_nc(t_steps=T):
    """Build the SPMD Bass program (identical on all cores)."""
    nc = bacc.Bacc("TRN2", target_bir_lowering=False, debug=False,
                   num_devices=NCORES)

    xT_d = nc.dram_tensor("xT", [D, t_steps * NB], BF16, kind="ExternalInput")
    wx_d = nc.dram_tensor("wx", [D, G], BF16, kind="ExternalInput")
    w2_d = nc.dram_tensor("w2", [2 * H, G], BF16, kind="ExternalInput")
    b128_d = nc.dram_tensor("b128", [128, G], BF16, kind="ExternalInput")
    at_d = nc.dram_tensor("at", [H, NB * P], BF16, kind="ExternalInput")
    atp_d = nc.dram_tensor("atp", [NB * P, H], BF16, kind="ExternalInput")
    h0T_d = nc.dram_tensor("h0T", [H, NB], BF16, kind="ExternalInput")
    h0q_d = nc.dram_tensor("h0q", [2 * 128, 128], F32, kind="ExternalInput")
    mask_d = nc.dram_tensor("mask", [128, 128], BF16, kind="ExternalInput")
    m4_d = nc.dram_tensor("m4", [128, 128], BF16, kind="ExternalInput")
    ident_d = nc.dram_tensor("ident", [128, 128], BF16, kind="ExternalInput")
    # quad-stacked bf16 output: row ((q*t + t)*128 + 32*gp + n), col c
    # holds h[n, t, (4q+gp)*128 + c]; host unshuffles + converts to f32
    out_d = nc.dram_tensor("out", [2 * t_steps * 128, 128], BF16,
                           kind="ExternalOutput")

    with tile.TileContext(nc) as tc:
        with tc.tile_pool(name="dram", bufs=1, space="DRAM") as dpool:
            # quad-stacked U: row (t*128 + gp*32 + n), col (q*512 + c)
            # holds U[t, n, gate block 4q+gp, c]
            u_dram = dpool.tile([t_steps * 128, 2 * 512], BF16)
            # bounce buffer for the softmax-weight partition scatter
            # layout: addr = p*128 + partition  (p-major)
            wdrT = dpool.tile([16, 128], BF16)

            with tc.tile_pool(name="res", bufs=1) as res, \
                 tc.tile_pool(name="ht", bufs=20) as htp, \
                 tc.tile_pool(name="u", bufs=2) as up, \
                 tc.tile_pool(name="st", bufs=2) as stp, \
                 tc.tile_pool(name="att", bufs=2) as attp, \
                 tc.tile_pool(name="blk", bufs=2) as blkp, \
                 tc.tile_pool(name="pax", bufs=8) as paxp, \
                 tc.tile_pool(name="paw", bufs=2) as pawp, \
                 tc.tile_pool(name="pab", bufs=2) as pabp, \
                 tc.tile_pool(name="pau", bufs=2) as pau, \
                 tc.tile_pool(name="psg", bufs=3, space="PSUM") as psg_p, \
                 tc.tile_pool(name="psa", bufs=2, space="PSUM") as pa_ps, \
                 tc.tile_pool(name="psx", bufs=1, space="PSUM") as psx_p:
                pl_ps = psx_p
                pt_ps = psx_p
                pss_p = psx_p

                # ---------------- phase A machinery ----------------
                # unit u = (mg, g, m): row-group mg, gate block g, local
                # row-tile m (global row-tile 4*mg + m). Order: mg, g, m.
                pa_xm = {}       # (mg, m) -> xm tile
                pa_wx = {}       # (mg, g) -> (wxg tile, b-slice tile)

                TNB = t_steps * NB

                def pa_fetch_xm(mg, m):
                    if (mg, m) in pa_xm or mg >= MG:
                        return
                    xm = paxp.tile([128, KH * 128], BF16, tag="xm")
                    rt = 4 * mg + m
                    # one DMA: dst (i, d, j) <- xT[128d + i, 128 rt + j]
                    b_ = xT_d[0:128, rt * 128:(rt + 1) * 128]
                    src = bass.AP(b_.tensor, b_.offset,
                                  [b_.ap[0], [128 * TNB, KH], b_.ap[1]])
                    nc.gpsimd.dma_start(
                        xm[:].rearrange("p (d j) -> p d j", d=KH), src)
                    pa_xm[(mg, m)] = xm

                def pa_fetch_unit(u):
                    mg, g = u // (GB * MPG), (u // MPG) % GB
                    if (mg, g) in pa_wx:
                        return
                    for m in range(MPG):       # xm for this group
                        pa_fetch_xm(mg, m)
                    if g >= 4:                 # trickle next group's xm
                        pa_fetch_xm(mg + 1, g - 4)
                    wxg = pawp.tile([128, KH * 512], BF16, tag="wxg")
                    b_ = wx_d[0:128, g * 512:(g + 1) * 512]
                    src = bass.AP(b_.tensor, b_.offset,
                                  [b_.ap[0], [128 * G, KH], b_.ap[1]])
                    nc.gpsimd.dma_start(
                        wxg[:].rearrange("p (d c) -> p d c", d=KH), src)
                    bsl = pabp.tile([128, 512], BF16, tag="bsl")
                    nc.gpsimd.dma_start(bsl[:], b128_d[:, g * 512:(g + 1) * 512])
                    pa_wx[(mg, g)] = (wxg, bsl)

                pa_pend = {}

                def pa_compute_mm(u):
                    # PE part only; finish (DVE add + store) issued later
                    mg, g, m = u // (GB * MPG), (u // MPG) % GB, u % MPG
                    wxg, bsl = pa_wx[(mg, g)]
                    xm = pa_xm[(mg, m)]
                    ps = pa_ps.tile([128, 512], F32, tag="ps")
                    for d_ in range(KH):
                        nc.tensor.matmul(ps[:], xm[:, d_ * 128:(d_ + 1) * 128],
                                         wxg[:, d_ * 512:(d_ + 1) * 512],
                                         start=(d_ == 0), stop=(d_ == KH - 1))
                    pa_pend[u] = (ps, bsl)

                def pa_compute_fin(u):
                    mg, g, m = u // (GB * MPG), (u // MPG) % GB, u % MPG
                    ps, bsl = pa_pend.pop(u)
                    us = pau.tile([128, 512], BF16, tag="us")
                    nc.vector.tensor_add(us[:], ps[:], bsl[:])
                    rt = 4 * mg + m
                    q, gp = divmod(g, 4)
                    # one DMA: us row 32r + n -> u_dram[(4rt+r)*128 + 32gp + n]
                    b_ = u_dram[4 * rt * 128 + gp * 32:
                                4 * rt * 128 + gp * 32 + 32,
                                q * 512:(q + 1) * 512]
                    dst = bass.AP(b_.tensor, b_.offset,
                                  [[128 * 1024, 4], b_.ap[0], b_.ap[1]])
                    nc.sync.dma_start(dst, us[:])
                    # free consumed group resources at unit boundaries
                    if m == MPG - 1 and g == GB - 1:
                        for mm in range(MPG):
                            pa_xm.pop((mg, mm), None)
                    if m == MPG - 1:
                        pa_wx.pop((mg, g), None)

                def pa_compute_unit(u):
                    pa_compute_mm(u)
                    pa_compute_fin(u)

                N_UNITS = MG * GB * MPG          # 128
                PRO_UNITS = GB * MPG             # group 0 upfront

                def pa_sched(t):
                    return range(min(PRO_UNITS + 2 * t, N_UNITS),
                                 min(PRO_UNITS + 2 * (t + 1), N_UNITS))

                # ---------------- static tiles ----------------
                w2 = []
                for k in range(K2):
                    t_ = res.tile([128, G], BF16, tag=f"w2_{k}")
                    nc.sync.dma_start(t_[:], w2_d[k * 128:(k + 1) * 128, :])
                    w2.append(t_)
                at_all = res.tile([128, KH * NB * P], BF16, tag="at_all")
                for k in range(KH):
                    nc.sync.dma_start(
                        at_all[:, k * NB * P:(k + 1) * NB * P],
                        at_d[k * 128:(k + 1) * 128, :])
                at = [at_all[:, k * NB * P:(k + 1) * NB * P]
                      for k in range(KH)]
                atp = []
                for j in range(NPC):
                    t_ = res.tile([128, H], BF16, tag=f"atp{j}")
                    nc.sync.dma_start(t_[:], atp_d[j * 128:(j + 1) * 128, :])
                    atp.append(t_)
                mask = res.tile([128, 128], BF16, tag="mask")
                nc.sync.dma_start(mask[:], mask_d[:])
                m4 = res.tile([128, 128], BF16, tag="m4")
                nc.sync.dma_start(m4[:], m4_d[:])
                ident = res.tile([128, 128], BF16, tag="ident")
                nc.sync.dma_start(ident[:], ident_d[:])
                w16rep = res.tile([128, 128], BF16, tag="w16rep")
                nc.vector.memset(w16rep[:], 0.0)

                # phase A prologue: group 0 complete (covers steps 0..15),
                # fetching one gate-block ahead of compute
                pa_fetch_unit(0)
                for u in range(PRO_UNITS):
                    pa_fetch_unit(u + MPG)
                    pa_compute_unit(u)
                for u in pa_sched(0):
                    pa_fetch_unit(u)

                def ht_slices(tq):
                    return [tq[k // 4][:, 32 * (k % 4):32 * (k % 4) + 32]
                            for k in range(KH)]

                hTq = []
                for q in range(2):
                    t_ = htp.tile([128, 128], BF16, tag="htq", bufs=6)
                    for gp in range(4):
                        k = 4 * q + gp
                        nc.sync.dma_start(t_[:, 32 * gp:32 * gp + 32],
                                          h0T_d[k * 128:(k + 1) * 128, :])
                    hTq.append(t_)
                hT = ht_slices(hTq)
                c_b = []
                for q in range(2):
                    t_ = blkp.tile([128, 128], F32, tag="c", bufs=4,
                                   name=f"c0_{q}")
                    nc.sync.dma_start(t_[:], h0q_d[q * 128:(q + 1) * 128, :])
                    c_b.append(t_)

                u_t = up.tile([128, 2 * 512], BF16, tag="u")
                nc.sync.dma_start(u_t[:], u_dram[0:128, :])

                inv_sqrt_h = 1.0 / math.sqrt(H)

                def gsl(q, gp):
                    g = 4 * q + gp
                    return slice(g * 512, (g + 1) * 512)

                def smm(pq, gp, lhs, rhs, stop):
                    # accumulate into the 32-row strip of the quad bank
                    if isinstance(lhs, tile.Tile):
                        lhs = lhs[:]
                    nc.tensor.matmul(pq[32 * gp:32 * gp + 32, :], lhs, rhs,
                                     start=False, stop=stop,
                                     tile_position=(0, 32 * gp),
                                     skip_group_check=True)

                def umm(pq, u, q):
                    # seed the whole quad bank with U via identity matmul
                    nc.tensor.matmul(pq[:], ident[:],
                                     u[:, q * 512:(q + 1) * 512],
                                     start=True, stop=False,
                                     skip_group_check=True)

                def score_mms(ps4, hTsl):
                    # scores in 4 col-tiled strips: strip j covers batches
                    # 8j..8j+8 (M=8 rows at partitions 32j..32j+8, N=128)
                    for k in range(KH):
                        for j in range(NPC):
                            nc.tensor.matmul(
                                ps4[32 * j:32 * j + 8, :],
                                hTsl[k][:, 8 * j:8 * j + 8],
                                at[k][:, 128 * j:128 * (j + 1)],
                                start=(k == 0), stop=(k == KH - 1),
                                tile_position=(0, 32 * j),
                                skip_group_check=True)

                # ---- prologue: scores S_0 + h-parts of both quads ----
                ps_s = pss_p.tile([128, 128], F32, tag="s")
                score_mms(ps_s, hT)
                pq0 = psg_p.tile([128, 512], F32, tag="g", name="pq0")
                umm(pq0, u_t, 0)
                for k in range(KH):
                    for gp in range(4):
                        smm(pq0, gp, hT[k], w2[k][:, gsl(0, gp)], False)
                pq1 = psg_p.tile([128, 512], F32, tag="g", name="pq1")
                umm(pq1, u_t, 1)
                for k in range(KH):
                    for gp in range(4):
                        smm(pq1, gp, hT[k], w2[k][:, gsl(1, gp)], False)

                for t in range(t_steps):
                    last = (t + 1 >= t_steps)
                    if not last:
                        u_next = up.tile([128, 2 * 512], BF16, tag="u")
                        nc.scalar.dma_start(
                            u_next[:], u_dram[(t + 1) * 128:(t + 2) * 128, :])

                    # PE filler for the softmax/scatter window
                    units = list(pa_sched(t))
                    if units:
                        pa_compute_mm(units[0])

                    # (a) softmax chain for step t (scores psum -> W16m).
                    # batch n = 8j+m lives at partition 32j+m throughout.
                    sm_sc = nc.enter_named_scope(f"sm{t}", False)
                    masked = stp.tile([128, 128], F32, tag="masked", bufs=1)
                    nc.vector.tensor_tensor(out=masked[:], in0=ps_s[:],
                                            in1=mask[:], op=ALU.mult)
                    sc = stp.tile([128, P], F32, tag="sc")
                    nc.vector.tensor_reduce(
                        sc[:], masked[:].rearrange("q (d p) -> q p d", p=P),
                        axis=AXX, op=ALU.add)
                    # exp(x) = s/(1-s) with s = sigmoid(x): keeps the ACT
                    # table cache at {Sigmoid, Tanh} with no per-step reloads
                    sg = stp.tile([128, P], F32, tag="sg")
                    nc.scalar.activation(sg[:], sc[:], AF.Sigmoid,
                                         scale=float(inv_sqrt_h))
                    om = stp.tile([128, P], F32, tag="om")
                    nc.scalar.activation(om[:], sc[:], AF.Sigmoid,
                                         scale=float(-inv_sqrt_h))
                    omr = stp.tile([128, P], F32, tag="omr")
                    nc.vector.reciprocal(omr[:], om[:])
                    expw = stp.tile([128, P], F32, tag="expw")
                    sume = stp.tile([128, 1], F32, tag="sume")
                    nc.vector.scalar_tensor_tensor(
                        out=expw[:], in0=sg[:], scalar=1.0, in1=omr[:],
                        op0=ALU.mult, op1=ALU.mult, accum_out=sume[:])
                    rec = stp.tile([128, 1], F32, tag="rec")
                    nc.vector.reciprocal(rec[:], sume[:])
                    w16p = stp.tile([128, P], BF16, tag="w16p")
                    nc.vector.tensor_scalar(out=w16p[:], in0=expw[:],
                                            scalar1=rec[:], scalar2=None,
                                            op0=ALU.mult)
                    # partition scatter via DRAM bounce (not SBUF-affine):
                    # push all partitions -> wdrT (batch n at row 32j+m),
                    # pull live rows replicated into the 8-col window of each
                    # block j; static m4 mask then zeroes all but the
                    # block-diagonal w[n,p]
                    wtd = wdrT[:]
                    push_dst = bass.AP(wtd.tensor, wtd.offset,
                                       [[1, 128], [128, P]])
                    nc.scalar.dma_start(push_dst, w16p[:])
                    rep_src = bass.AP(wtd.tensor, wtd.offset,
                                      [[0, 8], [128, P], [32, NPC], [1, 8]])
                    wr = w16rep[:]
                    rep_dst = bass.AP(wr.tensor, wr.offset,
                                      [wr.ap[0], [40, NPC], [1, 8]])
                    nc.scalar.dma_start(rep_dst, rep_src)
                    w16m = attp.tile([128, 128], BF16, tag="w16m")
                    nc.vector.tensor_tensor(out=w16m[:], in0=w16rep[:],
                                            in1=m4[:], op=ALU.mult)
                    nc.leave_named_scope(f"sm{t}", sm_sc[0], False)

                    # finish filler unit a now: its DVE add runs behind the
                    # softmax ops, freeing the psa bank well before the next
                    # step's filler matmuls need it
                    if units:
                        pa_compute_fin(units[0])

                    # (b) attention pooling on PE: attnT[k][c, n] =
                    #     sum_j atp_j[:, k]^T @ w16m[:, block j]
                    sc_ = nc.enter_named_scope(f"att{t}", False)
                    pool = pl_ps.tile([128, KH * 32], F32, tag="pool")
                    for k in range(KH):
                        for j in range(NPC):
                            nc.tensor.matmul(
                                pool[:, 32 * k:32 * k + 32],
                                atp[j][:, 128 * k:128 * (k + 1)],
                                w16m[:, 32 * j:32 * j + 32],
                                start=(j == 0), stop=(j == NPC - 1))
                    attn_sb = attp.tile([128, KH * 32], BF16, tag="attn_sb")
                    nc.vector.tensor_copy(attn_sb[:], pool[:])
                    attnT = [attn_sb[:, 32 * k:32 * k + 32]
                             for k in range(KH)]
                    nc.leave_named_scope(f"att{t}", sc_[0], False)

                    # per-step state: blocks 4q..4q+3 of quad q live on
                    # partitions 32g'..32g'+31 of the quad's PSUM bank
                    c_new = [blkp.tile([128, 128], F32, tag="c", bufs=4,
                                       name=f"cn{q}_{t}") for q in range(2)]
                    hbf = [blkp.tile([128, 128], BF16, tag="hbf",
                                     name=f"hbf{q}_{t}") for q in range(2)]

                    def quad_math(q, pq):
                        # all four blocks of the quad, read from PSUM
                        sio = blkp.tile([128, 384], F32, tag="sio")
                        nc.scalar.activation(sio[:], pq[:, 0:384], AF.Sigmoid)
                        tg = blkp.tile([128, 128], F32, tag="tg")
                        nc.scalar.activation(tg[:], pq[:, 384:512], AF.Tanh)
                        m1 = blkp.tile([128, 128], F32, tag="m1")
                        nc.vector.tensor_tensor(out=m1[:], in0=sio[:, 0:128],
                                                in1=tg[:], op=ALU.mult)
                        m2 = blkp.tile([128, 128], F32, tag="m2")
                        nc.vector.tensor_tensor(out=m2[:], in0=sio[:, 128:256],
                                                in1=c_b[q][:], op=ALU.mult)
                        nc.vector.tensor_add(c_new[q][:], m1[:], m2[:])
                        tcn = blkp.tile([128, 128], F32, tag="tcn")
                        nc.scalar.activation(tcn[:], c_new[q][:], AF.Tanh)
                        nc.vector.tensor_tensor(out=hbf[q][:],
                                                in0=sio[:, 256:384],
                                                in1=tcn[:], op=ALU.mult)
                        row = (q * t_steps + t) * 128
                        nc.sync.dma_start(out_d[row:row + 128, :], hbf[q][:])

                    # (f) attn-parts for quad 0 then quad0 math from PSUM
                    sc_ = nc.enter_named_scope(f"f05_{t}", False)
                    for k in range(KH, K2):
                        for gp in range(4):
                            smm(pq0, gp, attnT[k - KH], w2[k][:, gsl(0, gp)],
                                k == K2 - 1)
                    quad_math(0, pq0)
                    nc.leave_named_scope(f"f05_{t}", sc_[0], False)

                    # (h) attn-parts for quad 1 + quad1 math
                    sc_ = nc.enter_named_scope(f"h67_{t}", False)
                    for k in range(KH, K2):
                        for gp in range(4):
                            smm(pq1, gp, attnT[k - KH], w2[k][:, gsl(1, gp)],
                                k == K2 - 1)
                    quad_math(1, pq1)
                    nc.leave_named_scope(f"h67_{t}", sc_[0], False)

                    # phase-A filler inside the quad-math wait window
                    sc_ = nc.enter_named_scope(f"pa{t}", False)
                    if len(units) > 1:
                        pa_compute_mm(units[1])
                    nc.leave_named_scope(f"pa{t}", sc_[0], False)

                    # (i..l) PE transposes h->hT, next scores + h-parts
                    sc_ = nc.enter_named_scope(f"nxt{t}", False)
                    if not last:
                        hTq_new = []
                        for q in range(2):
                            tp = pt_ps.tile([128, 128], BF16, tag="tp")
                            nc.tensor.transpose(tp[:], hbf[q][:], ident[:])
                            ht_ = htp.tile([128, 128], BF16, tag="htq",
                                           bufs=6)
                            nc.vector.tensor_copy(ht_[:], tp[:])
                            hTq_new.append(ht_)
                        hT_new = ht_slices(hTq_new)
                        ps_s = pss_p.tile([128, 128], F32, tag="s")
                        score_mms(ps_s, hT_new)
                        pq0n = psg_p.tile([128, 512], F32, tag="g",
                                          name=f"pq0_{t}")
                        umm(pq0n, u_next, 0)
                        for k in range(KH):
                            for gp in range(4):
                                smm(pq0n, gp, hT_new[k],
                                    w2[k][:, gsl(0, gp)], False)
                        pq1n = psg_p.tile([128, 512], F32, tag="g",
                                          name=f"pq1_{t}")
                        umm(pq1n, u_next, 1)
                        for k in range(KH):
                            for gp in range(4):
                                smm(pq1n, gp, hT_new[k],
                                    w2[k][:, gsl(1, gp)], False)
                        pq0 = pq0n
                        pq1 = pq1n
                    nc.leave_named_scope(f"nxt{t}", sc_[0], False)

                    # phase-A unit-b finish + next step's fetches (late in
                    # the step so the DMA HW is quiet in the scatter window)
                    sc_ = nc.enter_named_scope(f"paf{t}", False)
                    for u in units[1:]:
                        pa_compute_fin(u)
                    for u in pa_sched(t + 1):
                        pa_fetch_unit(u)
                    nc.leave_named_scope(f"paf{t}", sc_[0], False)

                    if not last:
                        hT = hT_new
                        c_b = c_new
                        u_t = u_next

    nc.compile()
    return nc


def prepare_inputs(x, A, Wx, Wh, Wattn, b, t_steps=T):
    """Host-side sharding + layout prep. Returns list of per-core input maps."""
    x = np.asarray(x, dtype=np.float32)
    A = np.asarray(A, dtype=np.float32)
    Wx = np.asarray(Wx, dtype=np.float32)
    Wh = np.asarray(Wh, dtype=np.float32)
    Wattn = np.asarray(Wattn, dtype=np.float32)
    b = np.asarray(b, dtype=np.float32)

    perm = _gate_perm()
    wx_p = np.ascontiguousarray(Wx[:, perm]).astype(BF)
    w2_p = np.ascontiguousarray(np.vstack([Wh, Wattn])[:, perm]).astype(BF)
    b128 = np.ascontiguousarray(
        np.broadcast_to(b[perm], (128, G))).astype(BF)
    # scores diagonal-extraction mask: strip j holds batches 8j..8j+8 at
    # partitions 32j+m; batch 8j+m's own scores sit at cols m*16+p
    mask = np.zeros((128, 128), dtype=BF)
    for j in range(NPC):
        for m in range(8):
            mask[32 * j + m, m * P:(m + 1) * P] = 1
    # block-diagonal scatter mask: keeps (dn*16+p, 40j+dn) of the
    # replicated wT tile -> W16m[np', 32j + n] = w[n, p] for n = 8j+dn
    m4 = np.zeros((128, 128), dtype=BF)
    for j in range(NPC):
        for dn in range(8):
            for p in range(P):
                m4[dn * 16 + p, 40 * j + dn] = 1
    ident = np.eye(128, dtype=BF)
    in_maps = []
    for c in range(NCORES):
        x_c = x[c * NB:(c + 1) * NB, :t_steps]          # (NB, t, D)
        xr = x_c.transpose(1, 0, 2).reshape(t_steps * NB, D)  # t-major rows
        xT = np.ascontiguousarray(xr.T).astype(BF)       # (D, t*NB)
        A_c = A[c * NB:(c + 1) * NB].reshape(NB, H, P)
        at_c = np.ascontiguousarray(
            A_c.transpose(1, 0, 2).reshape(H, NB * P)).astype(BF)
        atp_c = np.ascontiguousarray(
            A_c.transpose(0, 2, 1).reshape(NB * P, H)).astype(BF)
        h0 = A_c.mean(axis=2).astype(np.float32)         # (NB, H)
        h0T = np.ascontiguousarray(h0.T).astype(BF)      # (H, NB)
        # quad-stacked initial cell state: block g of quad q lives on
        # partitions 32*(g%4), columns = h dims within the block
        h0q = np.empty((2 * 128, 128), dtype=np.float32)
        for g in range(8):
            q, gp = divmod(g, 4)
            h0q[q * 128 + gp * 32:q * 128 + (gp + 1) * 32, :] = \
                h0[:, g * 128:(g + 1) * 128]
        in_maps.append({
            "xT": xT, "wx": wx_p, "w2": w2_p, "b128": b128,
            "at": at_c, "atp": atp_c, "h0T": h0T, "h0q": h0q,
            "mask": mask, "m4": m4, "ident": ident,
        })
    return in_maps


def kernel(x, A, Wx, Wh, Wattn, b):
    from concourse.bass_utils import run_bass_kernel_spmd

    key = T
    if key not in _NC_CACHE:
        _NC_CACHE[key] = build_nc(T)
    nc = _NC_CACHE[key]

    in_maps = prepare_inputs(x, A, Wx, Wh, Wattn, b)
    trace = bool(int(os.environ.get("KERNEL_TRACE", "0")))
    tmpdir = os.environ.get("KERNEL_TRACE_DIR") or None
    res = run_bass_kernel_spmd(nc, in_maps, core_ids=list(range(NCORES)),
                               trace=trace, tmpdir=tmpdir)
    kernel.last_result = res
    if res.exec_time_ns is not None:
        print(f"HW exec time: {res.exec_time_ns} ns")
        kernel.last_exec_time_ns = res.exec_time_ns
    # unshuffle quad-stacked bf16 output: buf[q, t, gp, n, c] ->
    # h[n, t, (4q+gp)*128 + c]
    outs = []
    for r in res.results:
        buf = np.asarray(r["out"]).reshape(2, T, 4, NB, 128)
        outs.append(buf.transpose(3, 1, 0, 2, 4).reshape(NB, T, H))
    return np.concatenate(outs, axis=0).astype(np.float32)


kernel.last_exec_time_ns = None


# revision 34
# speedup vs baseline: 2.4614x; 2.4614x over previous
"""Trainium2 Bass kernel for an attention-augmented LSTM (CaptioningRNN).

Reference computation (per batch n, T timesteps):
    A_flat = A.reshape(N, H, 16); h0 = c0 = A_flat.mean(-1)
    scores_t = (h_{t-1} @ A_flat) / sqrt(H); w = softmax(scores)
    attn_t = A_flat @ w
    a = x_t @ Wx + h_{t-1} @ Wh + attn_t @ Wattn + b
    i, f, o, g = split(a, 4); c_t = sig(f)*c + sig(i)*tanh(g); h_t = sig(o)*tanh(c_t)

Strategy: data-parallel over batch across 8 cores (32 batch rows each).
Per core:
  Phase A: U = x @ Wx + b, streamed in row-group-blocked units (Wx read 4x
           instead of 16x), staged to DRAM bf16, paced 2 units/step into the
           recurrence's idle windows.
  Phase B: 64 recurrent steps. Gate matmul = [h; attn] (2048-dim contraction,
           bf16) against W2 = [Wh; Wattn] with gate-interleaved columns so each
           512-column block yields a full 128-dim slice of (i,f,o,g) and thus a
           128-dim slice of h/c. Attention scores on the tensor engine (all
           batch pairs, diagonal via mask + strided reduce). Attention POOLING
           also on the tensor engine: softmax weights are scattered into a
           block-diagonal [128np, 4x32n] operand (stream-transpose + replicated
           DMA + static mask) and contracted against a (n,p)-major copy of A
           (atp), yielding attnT chunks directly in [h,n] layout. h transposed
           back to hT layout with PE transpose-mode matmuls (not DMA).

Weight-matrix column order (gate interleave): block j (512 cols) holds
original columns [i|f|o|g][j*128:(j+1)*128]. The same permutation is applied
to Wx, b and hence U.
"""

import math
import os

import numpy as np
import ml_dtypes

import concourse.bass as bass
import concourse.mybir as mybir
import concourse.tile as tile
from concourse import bacc

N, T, D, H = 256, 64, 1024, 1024
NCORES = 8
NB = N // NCORES          # 32 batch rows per core
G = 4 * H                 # 4096 gate columns
P = 16                    # attention positions (4x4)
KH = H // 128             # 8 contraction chunks for h
K2 = (2 * H) // 128       # 16 contraction chunks for [h; attn]
GB = G // 512             # 8 gate blocks of 512
NPC = (NB * P) // 128     # 4 chunks of the (n,p) axis
F32 = mybir.dt.float32
BF16 = mybir.dt.bfloat16
BF = ml_dtypes.bfloat16

AF = mybir.ActivationFunctionType
ALU = mybir.AluOpType
AXX = mybir.AxisListType.X

_NC_CACHE = {}

# phase A row-group blocking: 4 groups x 4 row-tiles
MG = 4                    # row-tile groups
MPG = 4                   # row-tiles per group (each row-tile = 4 timesteps)
N_ROW_TILES = MG * MPG    # 16


def _gate_perm():
    """perm[new_col] = old_col for the gate-interleaved layout."""
    perm = np.empty(G, dtype=np.int64)
    for j in range(GB):
        for s in range(4):  # i, f, o, g
            perm[j * 512 + s * 128:(j * 512 + (s + 1) * 128)] = np.arange(
                s * H + j * 128, s * H + (j + 1) * 128)
    return perm


def build_nc(t_steps=T):
    """Build the SPMD Bass program (identical on all cores)."""
    nc = bacc.Bacc("TRN2", target_bir_lowering=False, debug=False,
                   num_devices=NCORES)

    xT_d = nc.dram_tensor("xT", [D, t_steps * NB], BF16, kind="ExternalInput")
    wx_d = nc.dram_tensor("wx", [D, G], BF16, kind="ExternalInput")
    w2_d = nc.dram_tensor("w2", [2 * H, G], BF16, kind="ExternalInput")
    b128_d = nc.dram_tensor("b128", [128, G], BF16, kind="ExternalInput")
    at_d = nc.dram_tensor("at", [H, NB * P], BF16, kind="ExternalInput")
    atp_d = nc.dram_tensor("atp", [NB * P, H], BF16, kind="ExternalInput")
    h0T_d = nc.dram_tensor("h0T", [H, NB], BF16, kind="ExternalInput")
    h0q_d = nc.dram_tensor("h0q", [2 * 128, 128], F32, kind="ExternalInput")
    mask_d = nc.dram_tensor("mask", [128, 128], BF16, kind="ExternalInput")
    m4_d = nc.dram_tensor("m4", [128, 128], BF16, kind="ExternalInput")
    esel_d = nc.dram_tensor("esel", [16, 128], BF16, kind="ExternalInput")
    ident_d = nc.dram_tensor("ident", [128, 128], BF16, kind="ExternalInput")
    # quad-stacked bf16 output: row ((q*t + t)*128 + 32*gp + n), col c
    # holds h[n, t, (4q+gp)*128 + c]; host unshuffles + converts to f32
    out_d = nc.dram_tensor("out", [2 * t_steps * 128, 128], BF16,
                           kind="ExternalOutput")

    with tile.TileContext(nc) as tc:
        with tc.tile_pool(name="dram", bufs=1, space="DRAM") as dpool:
            # quad-stacked U: row (t*128 + gp*32 + n), col (q*512 + c)
            # holds U[t, n, gate block 4q+gp, c]
            u_dram = dpool.tile([t_steps * 128, 2 * 512], BF16)


            with tc.tile_pool(name="res", bufs=1) as res, \
                 tc.tile_pool(name="ht", bufs=20) as htp, \
                 tc.tile_pool(name="u", bufs=2) as up, \
                 tc.tile_pool(name="st", bufs=2) as stp, \
                 tc.tile_pool(name="att", bufs=2) as attp, \
                 tc.tile_pool(name="blk", bufs=2) as blkp, \
                 tc.tile_pool(name="pax", bufs=8) as paxp, \
                 tc.tile_pool(name="paw", bufs=2) as pawp, \
                 tc.tile_pool(name="pab", bufs=2) as pabp, \
                 tc.tile_pool(name="pau", bufs=2) as pau, \
                 tc.tile_pool(name="psg", bufs=3, space="PSUM") as psg_p, \
                 tc.tile_pool(name="psa", bufs=2, space="PSUM") as pa_ps, \
                 tc.tile_pool(name="psx", bufs=1, space="PSUM") as psx_p:
                pl_ps = psx_p
                pt_ps = psx_p
                pss_p = psx_p

                # ---------------- phase A machinery ----------------
                # unit u = (mg, g, m): row-group mg, gate block g, local
                # row-tile m (global row-tile 4*mg + m). Order: mg, g, m.
                pa_xm = {}       # (mg, m) -> xm tile
                pa_wx = {}       # (mg, g) -> (wxg tile, b-slice tile)

                TNB = t_steps * NB

                def pa_fetch_xm(mg, m):
                    if (mg, m) in pa_xm or mg >= MG:
                        return
                    xm = paxp.tile([128, KH * 128], BF16, tag="xm")
                    rt = 4 * mg + m
                    # one DMA: dst (i, d, j) <- xT[128d + i, 128 rt + j]
                    b_ = xT_d[0:128, rt * 128:(rt + 1) * 128]
                    src = bass.AP(b_.tensor, b_.offset,
                                  [b_.ap[0], [128 * TNB, KH], b_.ap[1]])
                    nc.gpsimd.dma_start(
                        xm[:].rearrange("p (d j) -> p d j", d=KH), src)
                    pa_xm[(mg, m)] = xm

                def pa_fetch_unit(u):
                    mg, g = u // (GB * MPG), (u // MPG) % GB
                    if (mg, g) in pa_wx:
                        return
                    for m in range(MPG):       # xm for this group
                        pa_fetch_xm(mg, m)
                    if g >= 4:                 # trickle next group's xm
                        pa_fetch_xm(mg + 1, g - 4)
                    wxg = pawp.tile([128, KH * 512], BF16, tag="wxg")
                    b_ = wx_d[0:128, g * 512:(g + 1) * 512]
                    src = bass.AP(b_.tensor, b_.offset,
                                  [b_.ap[0], [128 * G, KH], b_.ap[1]])
                    nc.gpsimd.dma_start(
                        wxg[:].rearrange("p (d c) -> p d c", d=KH), src)
                    bsl = pabp.tile([128, 512], BF16, tag="bsl")
                    nc.gpsimd.dma_start(bsl[:], b128_d[:, g * 512:(g + 1) * 512])
                    pa_wx[(mg, g)] = (wxg, bsl)

                pa_pend = {}

                def pa_compute_mm(u):
                    # PE part only; finish (DVE add + store) issued later
                    mg, g, m = u // (GB * MPG), (u // MPG) % GB, u % MPG
                    wxg, bsl = pa_wx[(mg, g)]
                    xm = pa_xm[(mg, m)]
                    ps = pa_ps.tile([128, 512], F32, tag="ps")
                    for d_ in range(KH):
                        nc.tensor.matmul(ps[:], xm[:, d_ * 128:(d_ + 1) * 128],
                                         wxg[:, d_ * 512:(d_ + 1) * 512],
                                         start=(d_ == 0), stop=(d_ == KH - 1))
                    pa_pend[u] = (ps, bsl)

                def pa_compute_fin(u):
                    mg, g, m = u // (GB * MPG), (u // MPG) % GB, u % MPG
                    ps, bsl = pa_pend.pop(u)
                    us = pau.tile([128, 512], BF16, tag="us")
                    nc.vector.tensor_add(us[:], ps[:], bsl[:])
                    rt = 4 * mg + m
                    q, gp = divmod(g, 4)
                    # one DMA: us row 32r + n -> u_dram[(4rt+r)*128 + 32gp + n]
                    b_ = u_dram[4 * rt * 128 + gp * 32:
                                4 * rt * 128 + gp * 32 + 32,
                                q * 512:(q + 1) * 512]
                    dst = bass.AP(b_.tensor, b_.offset,
                                  [[128 * 1024, 4], b_.ap[0], b_.ap[1]])
                    nc.sync.dma_start(dst, us[:])
                    # free consumed group resources at unit boundaries
                    if m == MPG - 1 and g == GB - 1:
                        for mm in range(MPG):
                            pa_xm.pop((mg, mm), None)
                    if m == MPG - 1:
                        pa_wx.pop((mg, g), None)

                def pa_compute_unit(u):
                    pa_compute_mm(u)
                    pa_compute_fin(u)

                N_UNITS = MG * GB * MPG          # 128
                PRO_UNITS = GB * MPG             # group 0 upfront

                def pa_sched(t):
                    return range(min(PRO_UNITS + 2 * t, N_UNITS),
                                 min(PRO_UNITS + 2 * (t + 1), N_UNITS))

                # ---------------- static tiles ----------------
                w2 = []
                for k in range(K2):
                    t_ = res.tile([128, G], BF16, tag=f"w2_{k}")
                    nc.sync.dma_start(t_[:], w2_d[k * 128:(k + 1) * 128, :])
                    w2.append(t_)
                at_all = res.tile([128, KH * NB * P], BF16, tag="at_all")
                for k in range(KH):
                    nc.sync.dma_start(
                        at_all[:, k * NB * P:(k + 1) * NB * P],
                        at_d[k * 128:(k + 1) * 128, :])
                at = [at_all[:, k * NB * P:(k + 1) * NB * P]
                      for k in range(KH)]
                atp = []
                for j in range(NPC):
                    t_ = res.tile([128, H], BF16, tag=f"atp{j}")
                    nc.sync.dma_start(t_[:], atp_d[j * 128:(j + 1) * 128, :])
                    atp.append(t_)
                mask = res.tile([128, 128], BF16, tag="mask")
                nc.sync.dma_start(mask[:], mask_d[:])
                m4 = res.tile([128, 128], BF16, tag="m4")
                nc.sync.dma_start(m4[:], m4_d[:])
                esel = res.tile([16, 128], BF16, tag="esel")
                nc.sync.dma_start(esel[:], esel_d[:])
                ident = res.tile([128, 128], BF16, tag="ident")
                nc.sync.dma_start(ident[:], ident_d[:])

                # phase A prologue: group 0 complete (covers steps 0..15),
                # fetching one gate-block ahead of compute
                pa_fetch_unit(0)
                for u in range(PRO_UNITS):
                    pa_fetch_unit(u + MPG)
                    pa_compute_unit(u)
                for u in pa_sched(0):
                    pa_fetch_unit(u)

                def ht_slices(tq):
                    return [tq[k // 4][:, 32 * (k % 4):32 * (k % 4) + 32]
                            for k in range(KH)]

                hTq = []
                for q in range(2):
                    t_ = htp.tile([128, 128], BF16, tag="htq", bufs=6)
                    for gp in range(4):
                        k = 4 * q + gp
                        nc.sync.dma_start(t_[:, 32 * gp:32 * gp + 32],
                                          h0T_d[k * 128:(k + 1) * 128, :])
                    hTq.append(t_)
                hT = ht_slices(hTq)
                c_b = []
                for q in range(2):
                    t_ = blkp.tile([128, 128], F32, tag="c", bufs=4,
                                   name=f"c0_{q}")
                    nc.sync.dma_start(t_[:], h0q_d[q * 128:(q + 1) * 128, :])
                    c_b.append(t_)

                u_t = up.tile([128, 2 * 512], BF16, tag="u")
                nc.sync.dma_start(u_t[:], u_dram[0:128, :])

                inv_sqrt_h = 1.0 / math.sqrt(H)

                def gsl(q, gp):
                    g = 4 * q + gp
                    return slice(g * 512, (g + 1) * 512)

                def smm(pq, gp, lhs, rhs, stop):
                    # accumulate into the 32-row strip of the quad bank
                    if isinstance(lhs, tile.Tile):
                        lhs = lhs[:]
                    nc.tensor.matmul(pq[32 * gp:32 * gp + 32, :], lhs, rhs,
                                     start=False, stop=stop,
                                     tile_position=(0, 32 * gp),
                                     skip_group_check=True)

                def umm(pq, u, q):
                    # seed the whole quad bank with U via identity matmul
                    nc.tensor.matmul(pq[:], ident[:],
                                     u[:, q * 512:(q + 1) * 512],
                                     start=True, stop=False,
                                     skip_group_check=True)

                def score_mms(ps4, hTsl):
                    # scores in 4 col-tiled strips: strip j covers batches
                    # 8j..8j+8 (M=8 rows at partitions 32j..32j+8, N=128)
                    for k in range(KH):
                        for j in range(NPC):
                            nc.tensor.matmul(
                                ps4[32 * j:32 * j + 8, :],
                                hTsl[k][:, 8 * j:8 * j + 8],
                                at[k][:, 128 * j:128 * (j + 1)],
                                start=(k == 0), stop=(k == KH - 1),
                                tile_position=(0, 32 * j),
                                skip_group_check=True)

                # ---- prologue: scores S_0 + h-parts of both quads ----
                ps_s = pss_p.tile([128, 128], F32, tag="s")
                score_mms(ps_s, hT)
                pq0 = psg_p.tile([128, 512], F32, tag="g", name="pq0")
                umm(pq0, u_t, 0)
                for k in range(KH):
                    for gp in range(4):
                        smm(pq0, gp, hT[k], w2[k][:, gsl(0, gp)], False)
                pq1 = psg_p.tile([128, 512], F32, tag="g", name="pq1")
                umm(pq1, u_t, 1)
                for k in range(KH):
                    for gp in range(4):
                        smm(pq1, gp, hT[k], w2[k][:, gsl(1, gp)], False)

                for t in range(t_steps):
                    last = (t + 1 >= t_steps)
                    if not last:
                        u_next = up.tile([128, 2 * 512], BF16, tag="u")
                        nc.scalar.dma_start(
                            u_next[:], u_dram[(t + 1) * 128:(t + 2) * 128, :])

                    # PE filler for the softmax/scatter window
                    units = list(pa_sched(t))
                    if units:
                        pa_compute_mm(units[0])

                    # (a) softmax chain for step t (scores psum -> W16m).
                    # batch n = 8j+m lives at partition 32j+m throughout.
                    sm_sc = nc.enter_named_scope(f"sm{t}", False)
                    masked = stp.tile([128, 128], F32, tag="masked", bufs=1)
                    nc.vector.tensor_tensor(out=masked[:], in0=ps_s[:],
                                            in1=mask[:], op=ALU.mult)
                    sc = stp.tile([128, P], F32, tag="sc")
                    nc.vector.tensor_reduce(
                        sc[:], masked[:].rearrange("q (d p) -> q p d", p=P),
                        axis=AXX, op=ALU.add)
                    # exp(x) = s/(1-s) with s = sigmoid(x): keeps the ACT
                    # table cache at {Sigmoid, Tanh} with no per-step reloads
                    sg = stp.tile([128, P], F32, tag="sg")
                    nc.scalar.activation(sg[:], sc[:], AF.Sigmoid,
                                         scale=float(inv_sqrt_h))
                    om = stp.tile([128, P], F32, tag="om")
                    nc.scalar.activation(om[:], sc[:], AF.Sigmoid,
                                         scale=float(-inv_sqrt_h))
                    omr = stp.tile([128, P], F32, tag="omr")
                    nc.vector.reciprocal(omr[:], om[:])
                    expw = stp.tile([128, P], F32, tag="expw")
                    sume = stp.tile([128, 1], F32, tag="sume")
                    nc.vector.scalar_tensor_tensor(
                        out=expw[:], in0=sg[:], scalar=1.0, in1=omr[:],
                        op0=ALU.mult, op1=ALU.mult, accum_out=sume[:])
                    rec = stp.tile([128, 1], F32, tag="rec")
                    nc.vector.reciprocal(rec[:], sume[:])
                    w16p = stp.tile([128, P], BF16, tag="w16p")
                    nc.vector.tensor_scalar(out=w16p[:], in0=expw[:],
                                            scalar1=rec[:], scalar2=None,
                                            op0=ALU.mult)
                    # partition scatter fully on-chip: PE-transpose w16p to
                    # [16p, 128n], replicate across partitions with a single
                    # matmul against the static tiled identity (Esel^T @ wT
                    # gives out[q, col] = w[n(col), p(q)]), then the static
                    # m4 mask keeps only the block-diagonal scatter w[n,p]
                    wtp = pt_ps.tile([128, 128], BF16, tag="tp",
                                     name=f"wtp{t}")
                    nc.tensor.transpose(wtp[0:16, :], w16p[:], ident[:])
                    w16pT = attp.tile([16, 128], BF16, tag="w16pT")
                    nc.vector.tensor_copy(w16pT[:], wtp[0:16, :])
                    wrep = pss_p.tile([128, 128], F32, tag="s",
                                      name=f"wrep{t}")
                    nc.tensor.matmul(wrep[:], esel[:], w16pT[:],
                                     start=True, stop=True)
                    w16m = attp.tile([128, 128], BF16, tag="w16m")
                    nc.vector.tensor_tensor(out=w16m[:], in0=wrep[:],
                                            in1=m4[:], op=ALU.mult)
                    nc.leave_named_scope(f"sm{t}", sm_sc[0], False)

                    # finish filler unit a now: its DVE add runs behind the
                    # softmax ops, freeing the psa bank well before the next
                    # step's filler matmuls need it
                    if units:
                        pa_compute_fin(units[0])

                    # (b) attention pooling on PE: attnT[k][c, n] =
                    #     sum_j atp_j[:, k]^T @ w16m[:, block j]
                    sc_ = nc.enter_named_scope(f"att{t}", False)
                    # chunk j holds the (n,p) rows of batches 8j..8j+8, whose
                    # weights sit at w16m cols 32j..32j+8; each (k,j) matmul
                    # fills its own 8-col strip of attnT block k
                    pool = pl_ps.tile([128, KH * 32], F32, tag="pool")
                    for k in range(KH):
                        for j in range(NPC):
                            nc.tensor.matmul(
                                pool[:, 32 * k + 8 * j:32 * k + 8 * j + 8],
                                atp[j][:, 128 * k:128 * (k + 1)],
                                w16m[:, 32 * j:32 * j + 8],
                                start=True, stop=True)
                    attn_sb = attp.tile([128, KH * 32], BF16, tag="attn_sb")
                    nc.vector.tensor_copy(attn_sb[:], pool[:])
                    attnT = [attn_sb[:, 32 * k:32 * k + 32]
                             for k in range(KH)]
                    nc.leave_named_scope(f"att{t}", sc_[0], False)

                    # per-step state: blocks 4q..4q+3 of quad q live on
                    # partitions 32g'..32g'+31 of the quad's PSUM bank
                    c_new = [blkp.tile([128, 128], F32, tag="c", bufs=4,
                                       name=f"cn{q}_{t}") for q in range(2)]
                    hbf = [blkp.tile([128, 128], BF16, tag="hbf",
                                     name=f"hbf{q}_{t}") for q in range(2)]

                    def quad_math(q, pq):
                        # all four blocks of the quad, read from PSUM
                        sio = blkp.tile([128, 384], F32, tag="sio")
                        nc.scalar.activation(sio[:], pq[:, 0:384], AF.Sigmoid)
                        tg = blkp.tile([128, 128], F32, tag="tg")
                        nc.scalar.activation(tg[:], pq[:, 384:512], AF.Tanh)
                        m1 = blkp.tile([128, 128], F32, tag="m1")
                        nc.vector.tensor_tensor(out=m1[:], in0=sio[:, 0:128],
                                                in1=tg[:], op=ALU.mult)
                        m2 = blkp.tile([128, 128], F32, tag="m2")
                        nc.vector.tensor_tensor(out=m2[:], in0=sio[:, 128:256],
                                                in1=c_b[q][:], op=ALU.mult)
                        nc.vector.tensor_add(c_new[q][:], m1[:], m2[:])
                        tcn = blkp.tile([128, 128], F32, tag="tcn")
                        nc.scalar.activation(tcn[:], c_new[q][:], AF.Tanh)
                        nc.vector.tensor_tensor(out=hbf[q][:],
                                                in0=sio[:, 256:384],
                                                in1=tcn[:], op=ALU.mult)
                        row = (q * t_steps + t) * 128
                        nc.sync.dma_start(out_d[row:row + 128, :], hbf[q][:])

                    # (f) attn-parts for quad 0 then quad0 math from PSUM
                    sc_ = nc.enter_named_scope(f"f05_{t}", False)
                    for k in range(KH, K2):
                        for gp in range(4):
                            smm(pq0, gp, attnT[k - KH], w2[k][:, gsl(0, gp)],
                                k == K2 - 1)
                    quad_math(0, pq0)
                    nc.leave_named_scope(f"f05_{t}", sc_[0], False)

                    # (h) attn-parts for quad 1 + quad1 math
                    sc_ = nc.enter_named_scope(f"h67_{t}", False)
                    for k in range(KH, K2):
                        for gp in range(4):
                            smm(pq1, gp, attnT[k - KH], w2[k][:, gsl(1, gp)],
                                k == K2 - 1)
                    quad_math(1, pq1)
                    nc.leave_named_scope(f"h67_{t}", sc_[0], False)

                    # phase-A filler inside the quad-math wait window
                    sc_ = nc.enter_named_scope(f"pa{t}", False)
                    if len(units) > 1:
                        pa_compute_mm(units[1])
                    nc.leave_named_scope(f"pa{t}", sc_[0], False)

                    # (i..l) PE transposes h->hT, next scores + h-parts
                    sc_ = nc.enter_named_scope(f"nxt{t}", False)
                    if not last:
                        hTq_new = []
                        for q in range(2):
                            tp = pt_ps.tile([128, 128], BF16, tag="tp")
                            nc.tensor.transpose(tp[:], hbf[q][:], ident[:])
                            ht_ = htp.tile([128, 128], BF16, tag="htq",
                                           bufs=6)
                            nc.vector.tensor_copy(ht_[:], tp[:])
                            hTq_new.append(ht_)
                        hT_new = ht_slices(hTq_new)
                        ps_s = pss_p.tile([128, 128], F32, tag="s")
                        score_mms(ps_s, hT_new)
                        pq0n = psg_p.tile([128, 512], F32, tag="g",
                                          name=f"pq0_{t}")
                        umm(pq0n, u_next, 0)
                        for k in range(KH):
                            for gp in range(4):
                                smm(pq0n, gp, hT_new[k],
                                    w2[k][:, gsl(0, gp)], False)
                        pq1n = psg_p.tile([128, 512], F32, tag="g",
                                          name=f"pq1_{t}")
                        umm(pq1n, u_next, 1)
                        for k in range(KH):
                            for gp in range(4):
                                smm(pq1n, gp, hT_new[k],
                                    w2[k][:, gsl(1, gp)], False)
                        pq0 = pq0n
                        pq1 = pq1n
                    nc.leave_named_scope(f"nxt{t}", sc_[0], False)

                    # phase-A unit-b finish + next step's fetches (late in
                    # the step so the DMA HW is quiet in the scatter window)
                    sc_ = nc.enter_named_scope(f"paf{t}", False)
                    for u in units[1:]:
                        pa_compute_fin(u)
                    for u in pa_sched(t + 1):
                        pa_fetch_unit(u)
                    nc.leave_named_scope(f"paf{t}", sc_[0], False)

                    if not last:
                        hT = hT_new
                        c_b = c_new
                        u_t = u_next

    nc.compile()
    return nc


def prepare_inputs(x, A, Wx, Wh, Wattn, b, t_steps=T):
    """Host-side sharding + layout prep. Returns list of per-core input maps."""
    x = np.asarray(x, dtype=np.float32)
    A = np.asarray(A, dtype=np.float32)
    Wx = np.asarray(Wx, dtype=np.float32)
    Wh = np.asarray(Wh, dtype=np.float32)
    Wattn = np.asarray(Wattn, dtype=np.float32)
    b = np.asarray(b, dtype=np.float32)

    perm = _gate_perm()
    wx_p = np.ascontiguousarray(Wx[:, perm]).astype(BF)
    w2_p = np.ascontiguousarray(np.vstack([Wh, Wattn])[:, perm]).astype(BF)
    b128 = np.ascontiguousarray(
        np.broadcast_to(b[perm], (128, G))).astype(BF)
    # scores diagonal-extraction mask: strip j holds batches 8j..8j+8 at
    # partitions 32j+m; batch 8j+m's own scores sit at cols m*16+p
    mask = np.zeros((128, 128), dtype=BF)
    for j in range(NPC):
        for m in range(8):
            mask[32 * j + m, m * P:(m + 1) * P] = 1
    # block-diagonal scatter mask: keeps (dn*16+p, 40j+dn) of the
    # replicated wT tile -> W16m[np', 32j + n] = w[n, p] for n = 8j+dn
    # pool-scatter mask: chunk j's (dn,p) row keeps batch 8j+dn, which the
    # Esel-replicated tile holds at col 32j+dn
    m4 = np.zeros((128, 128), dtype=BF)
    for j in range(NPC):
        for dn in range(8):
            for p in range(P):
                m4[dn * 16 + p, 32 * j + dn] = 1
    esel = np.tile(np.eye(P, dtype=BF), (1, 8))
    ident = np.eye(128, dtype=BF)
    in_maps = []
    for c in range(NCORES):
        x_c = x[c * NB:(c + 1) * NB, :t_steps]          # (NB, t, D)
        xr = x_c.transpose(1, 0, 2).reshape(t_steps * NB, D)  # t-major rows
        xT = np.ascontiguousarray(xr.T).astype(BF)       # (D, t*NB)
        A_c = A[c * NB:(c + 1) * NB].reshape(NB, H, P)
        at_c = np.ascontiguousarray(
            A_c.transpose(1, 0, 2).reshape(H, NB * P)).astype(BF)
        atp_c = np.ascontiguousarray(
            A_c.transpose(0, 2, 1).reshape(NB * P, H)).astype(BF)
        h0 = A_c.mean(axis=2).astype(np.float32)         # (NB, H)
        h0T = np.ascontiguousarray(h0.T).astype(BF)      # (H, NB)
        # quad-stacked initial cell state: block g of quad q lives on
        # partitions 32*(g%4), columns = h dims within the block
        h0q = np.empty((2 * 128, 128), dtype=np.float32)
        for g in range(8):
            q, gp = divmod(g, 4)
            h0q[q * 128 + gp * 32:q * 128 + (gp + 1) * 32, :] = \
                h0[:, g * 128:(g + 1) * 128]
        in_maps.append({
            "xT": xT, "wx": wx_p, "w2": w2_p, "b128": b128,
            "at": at_c, "atp": atp_c, "h0T": h0T, "h0q": h0q,
            "mask": mask, "m4": m4, "esel": esel, "ident": ident,
        })
    return in_maps


def kernel(x, A, Wx, Wh, Wattn, b):
    from concourse.bass_utils import run_bass_kernel_spmd

    key = T
    if key not in _NC_CACHE:
        _NC_CACHE[key] = build_nc(T)
    nc = _NC_CACHE[key]

    in_maps = prepare_inputs(x, A, Wx, Wh, Wattn, b)
    trace = bool(int(os.environ.get("KERNEL_TRACE", "0")))
    tmpdir = os.environ.get("KERNEL_TRACE_DIR") or None
    res = run_bass_kernel_spmd(nc, in_maps, core_ids=list(range(NCORES)),
                               trace=trace, tmpdir=tmpdir)
    kernel.last_result = res
    if res.exec_time_ns is not None:
        print(f"HW exec time: {res.exec_time_ns} ns")
        kernel.last_exec_time_ns = res.exec_time_ns
    # unshuffle quad-stacked bf16 output: buf[q, t, gp, n, c] ->
    # h[n, t, (4q+gp)*128 + c]
    outs = []
    for r in res.results:
        buf = np.asarray(r["out"]).reshape(2, T, 4, NB, 128)
        outs.append(buf.transpose(3, 1, 0, 2, 4).reshape(NB, T, H))
    return np.concatenate(outs, axis=0).astype(np.float32)


kernel.last_exec_time_ns = None


# revision 42
# speedup vs baseline: 2.5526x; 1.0371x over previous
"""Trainium2 Bass kernel for an attention-augmented LSTM (CaptioningRNN).

Reference computation (per batch n, T timesteps):
    A_flat = A.reshape(N, H, 16); h0 = c0 = A_flat.mean(-1)
    scores_t = (h_{t-1} @ A_flat) / sqrt(H); w = softmax(scores)
    attn_t = A_flat @ w
    a = x_t @ Wx + h_{t-1} @ Wh + attn_t @ Wattn + b
    i, f, o, g = split(a, 4); c_t = sig(f)*c + sig(i)*tanh(g); h_t = sig(o)*tanh(c_t)

Strategy: data-parallel over batch across 8 cores (32 batch rows each).
Per core:
  Phase A: U = x @ Wx + b, streamed in row-group-blocked units (Wx read 4x
           instead of 16x), staged to DRAM bf16, paced 2 units/step into the
           recurrence's idle windows.
  Phase B: 64 recurrent steps. Gate matmul = [h; attn] (2048-dim contraction,
           bf16) against W2 = [Wh; Wattn] with gate-interleaved columns so each
           512-column block yields a full 128-dim slice of (i,f,o,g) and thus a
           128-dim slice of h/c. Attention scores on the tensor engine (all
           batch pairs, diagonal via mask + strided reduce). Attention POOLING
           also on the tensor engine: softmax weights are scattered into a
           block-diagonal [128np, 4x32n] operand (stream-transpose + replicated
           DMA + static mask) and contracted against a (n,p)-major copy of A
           (atp), yielding attnT chunks directly in [h,n] layout. h transposed
           back to hT layout with PE transpose-mode matmuls (not DMA).

Weight-matrix column order (gate interleave): block j (512 cols) holds
original columns [i|f|o|g][j*128:(j+1)*128]. The same permutation is applied
to Wx, b and hence U.
"""

import math
import os

import numpy as np
import ml_dtypes

import concourse.bass as bass
import concourse.mybir as mybir
import concourse.tile as tile
from concourse import bacc

N, T, D, H = 256, 64, 1024, 1024
NCORES = 8
NB = N // NCORES          # 32 batch rows per core
G = 4 * H                 # 4096 gate columns
P = 16                    # attention positions (4x4)
KH = H // 128             # 8 contraction chunks for h
K2 = (2 * H) // 128       # 16 contraction chunks for [h; attn]
GB = G // 512             # 8 gate blocks of 512
NPC = (NB * P) // 128     # 4 chunks of the (n,p) axis
F32 = mybir.dt.float32
BF16 = mybir.dt.bfloat16
BF = ml_dtypes.bfloat16

AF = mybir.ActivationFunctionType
ALU = mybir.AluOpType
AXX = mybir.AxisListType.X

_NC_CACHE = {}

# phase A row-group blocking: 4 groups x 4 row-tiles
MG = 4                    # row-tile groups
MPG = 4                   # row-tiles per group (each row-tile = 4 timesteps)
N_ROW_TILES = MG * MPG    # 16


def _gate_perm():
    """perm[new_col] = old_col for the gate-interleaved layout."""
    perm = np.empty(G, dtype=np.int64)
    for j in range(GB):
        for s in range(4):  # i, f, o, g
            perm[j * 512 + s * 128:(j * 512 + (s + 1) * 128)] = np.arange(
                s * H + j * 128, s * H + (j + 1) * 128)
    return perm


def build_nc(t_steps=T):
    """Build the SPMD Bass program (identical on all cores)."""
    nc = bacc.Bacc("TRN2", target_bir_lowering=False, debug=False,
                   num_devices=NCORES)

    xT_d = nc.dram_tensor("xT", [D, t_steps * NB], BF16, kind="ExternalInput")
    wx_d = nc.dram_tensor("wx", [D, G], BF16, kind="ExternalInput")
    w2_d = nc.dram_tensor("w2", [2 * H, G], BF16, kind="ExternalInput")
    b128_d = nc.dram_tensor("b128", [128, G], BF16, kind="ExternalInput")
    at_d = nc.dram_tensor("at", [H, NB * P], BF16, kind="ExternalInput")
    atp_d = nc.dram_tensor("atp", [NB * P, H], BF16, kind="ExternalInput")
    h0T_d = nc.dram_tensor("h0T", [H, NB], BF16, kind="ExternalInput")
    h0q_d = nc.dram_tensor("h0q", [2 * 128, 128], F32, kind="ExternalInput")
    mask_d = nc.dram_tensor("mask", [128, 128], BF16, kind="ExternalInput")
    m4_d = nc.dram_tensor("m4", [128, 128], BF16, kind="ExternalInput")
    esel_d = nc.dram_tensor("esel", [16, 128], BF16, kind="ExternalInput")
    ident_d = nc.dram_tensor("ident", [128, 128], BF16, kind="ExternalInput")
    # quad-stacked bf16 output: row ((q*t + t)*128 + 32*gp + n), col c
    # holds h[n, t, (4q+gp)*128 + c]; host unshuffles + converts to f32
    out_d = nc.dram_tensor("out", [2 * t_steps * 128, 128], BF16,
                           kind="ExternalOutput")

    with tile.TileContext(nc) as tc:
        with tc.tile_pool(name="dram", bufs=1, space="DRAM") as dpool:
            # quad-stacked U: row (t*128 + gp*32 + n), col (q*512 + c)
            # holds U[t, n, gate block 4q+gp, c]
            u_dram = dpool.tile([t_steps * 128, 2 * 512], BF16)


            with tc.tile_pool(name="res", bufs=1) as res, \
                 tc.tile_pool(name="ht", bufs=20) as htp, \
                 tc.tile_pool(name="u", bufs=2) as up, \
                 tc.tile_pool(name="st", bufs=2) as stp, \
                 tc.tile_pool(name="att", bufs=2) as attp, \
                 tc.tile_pool(name="blk", bufs=2) as blkp, \
                 tc.tile_pool(name="pax", bufs=8) as paxp, \
                 tc.tile_pool(name="paw", bufs=2) as pawp, \
                 tc.tile_pool(name="pab", bufs=2) as pabp, \
                 tc.tile_pool(name="pau", bufs=2) as pau, \
                 tc.tile_pool(name="psg", bufs=3, space="PSUM") as psg_p, \
                 tc.tile_pool(name="psa", bufs=2, space="PSUM") as pa_ps, \
                 tc.tile_pool(name="psx", bufs=1, space="PSUM") as psx_p:
                pl_ps = psx_p
                pt_ps = psx_p
                pss_p = psx_p

                # ---------------- phase A machinery ----------------
                # unit u = (mg, g, m): row-group mg, gate block g, local
                # row-tile m (global row-tile 4*mg + m). Order: mg, g, m.
                pa_xm = {}       # (mg, m) -> xm tile
                pa_wx = {}       # (mg, g) -> (wxg tile, b-slice tile)

                TNB = t_steps * NB

                def pa_fetch_xm(mg, m):
                    if (mg, m) in pa_xm or mg >= MG:
                        return
                    xm = paxp.tile([128, KH * 128], BF16, tag="xm")
                    rt = 4 * mg + m
                    # one DMA: dst (i, d, j) <- xT[128d + i, 128 rt + j]
                    b_ = xT_d[0:128, rt * 128:(rt + 1) * 128]
                    src = bass.AP(b_.tensor, b_.offset,
                                  [b_.ap[0], [128 * TNB, KH], b_.ap[1]])
                    nc.gpsimd.dma_start(
                        xm[:].rearrange("p (d j) -> p d j", d=KH), src)
                    pa_xm[(mg, m)] = xm

                def pa_fetch_unit(u):
                    mg, g = u // (GB * MPG), (u // MPG) % GB
                    if (mg, g) in pa_wx:
                        return
                    for m in range(MPG):       # xm for this group
                        pa_fetch_xm(mg, m)
                    if g >= 4:                 # trickle next group's xm
                        pa_fetch_xm(mg + 1, g - 4)
                    wxg = pawp.tile([128, KH * 512], BF16, tag="wxg")
                    b_ = wx_d[0:128, g * 512:(g + 1) * 512]
                    src = bass.AP(b_.tensor, b_.offset,
                                  [b_.ap[0], [128 * G, KH], b_.ap[1]])
                    nc.gpsimd.dma_start(
                        wxg[:].rearrange("p (d c) -> p d c", d=KH), src)
                    bsl = pabp.tile([128, 512], BF16, tag="bsl")
                    nc.gpsimd.dma_start(bsl[:], b128_d[:, g * 512:(g + 1) * 512])
                    pa_wx[(mg, g)] = (wxg, bsl)

                pa_pend = {}

                def pa_compute_mm(u):
                    # PE part only; finish (DVE add + store) issued later
                    mg, g, m = u // (GB * MPG), (u // MPG) % GB, u % MPG
                    wxg, bsl = pa_wx[(mg, g)]
                    xm = pa_xm[(mg, m)]
                    ps = pa_ps.tile([128, 512], F32, tag="ps")
                    for d_ in range(KH):
                        nc.tensor.matmul(ps[:], xm[:, d_ * 128:(d_ + 1) * 128],
                                         wxg[:, d_ * 512:(d_ + 1) * 512],
                                         start=(d_ == 0), stop=(d_ == KH - 1))
                    pa_pend[u] = (ps, bsl)

                def pa_compute_fin(u):
                    mg, g, m = u // (GB * MPG), (u // MPG) % GB, u % MPG
                    ps, bsl = pa_pend.pop(u)
                    us = pau.tile([128, 512], BF16, tag="us")
                    nc.vector.tensor_add(us[:], ps[:], bsl[:])
                    rt = 4 * mg + m
                    q, gp = divmod(g, 4)
                    # one DMA: us row 32r + n -> u_dram[(4rt+r)*128 + 32gp + n]
                    b_ = u_dram[4 * rt * 128 + gp * 32:
                                4 * rt * 128 + gp * 32 + 32,
                                q * 512:(q + 1) * 512]
                    dst = bass.AP(b_.tensor, b_.offset,
                                  [[128 * 1024, 4], b_.ap[0], b_.ap[1]])
                    nc.sync.dma_start(dst, us[:])
                    # free consumed group resources at unit boundaries
                    if m == MPG - 1 and g == GB - 1:
                        for mm in range(MPG):
                            pa_xm.pop((mg, mm), None)
                    if m == MPG - 1:
                        pa_wx.pop((mg, g), None)

                def pa_compute_unit(u):
                    pa_compute_mm(u)
                    pa_compute_fin(u)

                N_UNITS = MG * GB * MPG          # 128
                PRO_UNITS = GB * MPG             # group 0 upfront

                def pa_sched(t):
                    # 2 filler units/step; exhausted by step 48 (group 3's
                    # rows are needed at step 48, so later pacing is unsound)
                    return range(min(PRO_UNITS + 2 * t, N_UNITS),
                                 min(PRO_UNITS + 2 * (t + 1), N_UNITS))

                # ---------------- static tiles ----------------
                # spread the 16 MB w2 load across four queues so startup
                # isn't serialized on one DMA queue
                qs = [nc.sync, nc.scalar, nc.gpsimd]
                w2 = []
                for k in range(K2):
                    t_ = res.tile([128, G], BF16, tag=f"w2_{k}")
                    qs[k % 3].dma_start(t_[:], w2_d[k * 128:(k + 1) * 128, :])
                    w2.append(t_)
                at_all = res.tile([128, KH * NB * P], BF16, tag="at_all")
                for k in range(KH):
                    nc.sync.dma_start(
                        at_all[:, k * NB * P:(k + 1) * NB * P],
                        at_d[k * 128:(k + 1) * 128, :])
                at = [at_all[:, k * NB * P:(k + 1) * NB * P]
                      for k in range(KH)]
                atp = []
                for j in range(NPC):
                    t_ = res.tile([128, H], BF16, tag=f"atp{j}")
                    nc.sync.dma_start(t_[:], atp_d[j * 128:(j + 1) * 128, :])
                    atp.append(t_)
                mask = res.tile([128, 128], BF16, tag="mask")
                nc.sync.dma_start(mask[:], mask_d[:])
                m4 = res.tile([128, 128], BF16, tag="m4")
                nc.sync.dma_start(m4[:], m4_d[:])
                esel = res.tile([16, 128], BF16, tag="esel")
                nc.sync.dma_start(esel[:], esel_d[:])
                ident = res.tile([128, 128], BF16, tag="ident")
                nc.sync.dma_start(ident[:], ident_d[:])

                # phase A prologue: group 0 complete (covers steps 0..15),
                # fetching one gate-block ahead of compute
                pa_fetch_unit(0)
                for u in range(PRO_UNITS):
                    pa_fetch_unit(u + MPG)
                    pa_compute_unit(u)
                for u in pa_sched(0):
                    pa_fetch_unit(u)

                def ht_slices(tq):
                    return [tq[k // 4][:, 32 * (k % 4):32 * (k % 4) + 32]
                            for k in range(KH)]

                hTq = []
                for q in range(2):
                    t_ = htp.tile([128, 128], BF16, tag="htq", bufs=6)
                    for gp in range(4):
                        k = 4 * q + gp
                        nc.sync.dma_start(t_[:, 32 * gp:32 * gp + 32],
                                          h0T_d[k * 128:(k + 1) * 128, :])
                    hTq.append(t_)
                hT = ht_slices(hTq)
                c_b = []
                for q in range(2):
                    t_ = blkp.tile([128, 128], F32, tag="c", bufs=4,
                                   name=f"c0_{q}")
                    nc.sync.dma_start(t_[:], h0q_d[q * 128:(q + 1) * 128, :])
                    c_b.append(t_)

                u_t = up.tile([128, 2 * 512], BF16, tag="u")
                nc.sync.dma_start(u_t[:], u_dram[0:128, :])

                inv_sqrt_h = 1.0 / math.sqrt(H)

                def gsl(q, gp):
                    g = 4 * q + gp
                    return slice(g * 512, (g + 1) * 512)

                def smm(pq, gp, lhs, rhs, stop):
                    # accumulate into the 32-row strip of the quad bank
                    if isinstance(lhs, tile.Tile):
                        lhs = lhs[:]
                    nc.tensor.matmul(pq[32 * gp:32 * gp + 32, :], lhs, rhs,
                                     start=False, stop=stop,
                                     tile_position=(0, 32 * gp),
                                     skip_group_check=True)

                def umm(pq, u, q):
                    # seed the whole quad bank with U via identity matmul
                    nc.tensor.matmul(pq[:], ident[:],
                                     u[:, q * 512:(q + 1) * 512],
                                     start=True, stop=False,
                                     skip_group_check=True)

                def score_mms(ps4, hTsl):
                    # scores in 4 col-tiled strips: strip j covers batches
                    # 8j..8j+8 (M=8 rows at partitions 32j..32j+8, N=128)
                    for k in range(KH):
                        for j in range(NPC):
                            nc.tensor.matmul(
                                ps4[32 * j:32 * j + 8, :],
                                hTsl[k][:, 8 * j:8 * j + 8],
                                at[k][:, 128 * j:128 * (j + 1)],
                                start=(k == 0), stop=(k == KH - 1),
                                tile_position=(0, 32 * j),
                                skip_group_check=True)

                # ---- prologue: scores S_0 + h-parts of both quads ----
                ps_s = pss_p.tile([128, 128], F32, tag="s")
                score_mms(ps_s, hT)
                pq0 = psg_p.tile([128, 512], F32, tag="g", name="pq0")
                umm(pq0, u_t, 0)
                for k in range(KH):
                    for gp in range(4):
                        smm(pq0, gp, hT[k], w2[k][:, gsl(0, gp)], False)
                pq1 = psg_p.tile([128, 512], F32, tag="g", name="pq1")
                umm(pq1, u_t, 1)
                for k in range(KH):
                    for gp in range(4):
                        smm(pq1, gp, hT[k], w2[k][:, gsl(1, gp)], False)

                for t in range(t_steps):
                    last = (t + 1 >= t_steps)
                    if not last:
                        u_next = up.tile([128, 2 * 512], BF16, tag="u")
                        nc.scalar.dma_start(
                            u_next[:], u_dram[(t + 1) * 128:(t + 2) * 128, :])

                    # PE filler for the softmax/scatter window
                    units = list(pa_sched(t))
                    if units:
                        pa_compute_mm(units[0])

                    # (a) softmax chain for step t (scores psum -> W16m).
                    # batch n = 8j+m lives at partition 32j+m throughout.
                    sm_sc = nc.enter_named_scope(f"sm{t}", False)
                    masked = stp.tile([128, 128], F32, tag="masked", bufs=1)
                    nc.vector.tensor_tensor(out=masked[:], in0=ps_s[:],
                                            in1=mask[:], op=ALU.mult)
                    sc = stp.tile([128, P], F32, tag="sc")
                    nc.vector.tensor_reduce(
                        sc[:], masked[:].rearrange("q (d p) -> q p d", p=P),
                        axis=AXX, op=ALU.add)
                    # exp(x) = s/(1-s) with s = sigmoid(x): keeps the ACT
                    # table cache at {Sigmoid, Tanh} with no per-step reloads
                    sg = stp.tile([128, P], F32, tag="sg")
                    nc.scalar.activation(sg[:], sc[:], AF.Sigmoid,
                                         scale=float(inv_sqrt_h))
                    om = stp.tile([128, P], F32, tag="om")
                    nc.scalar.activation(om[:], sc[:], AF.Sigmoid,
                                         scale=float(-inv_sqrt_h))
                    omr = stp.tile([128, P], F32, tag="omr")
                    nc.vector.reciprocal(omr[:], om[:])
                    expw = stp.tile([128, P], F32, tag="expw")
                    sume = stp.tile([128, 1], F32, tag="sume")
                    nc.vector.scalar_tensor_tensor(
                        out=expw[:], in0=sg[:], scalar=1.0, in1=omr[:],
                        op0=ALU.mult, op1=ALU.mult, accum_out=sume[:])
                    rec = stp.tile([128, 1], F32, tag="rec")
                    nc.vector.reciprocal(rec[:], sume[:])
                    w16p = stp.tile([128, P], BF16, tag="w16p")
                    nc.vector.tensor_scalar(out=w16p[:], in0=expw[:],
                                            scalar1=rec[:], scalar2=None,
                                            op0=ALU.mult)
                    # partition scatter fully on-chip: PE-transpose w16p to
                    # [16p, 128n], replicate across partitions with a single
                    # matmul against the static tiled identity (Esel^T @ wT
                    # gives out[q, col] = w[n(col), p(q)]), then the static
                    # m4 mask keeps only the block-diagonal scatter w[n,p]
                    wtp = pt_ps.tile([128, 128], BF16, tag="tp",
                                     name=f"wtp{t}")
                    nc.tensor.transpose(wtp[0:16, :], w16p[:], ident[:])
                    w16pT = attp.tile([16, 128], BF16, tag="w16pT")
                    nc.vector.tensor_copy(w16pT[:], wtp[0:16, :])
                    wrep = pss_p.tile([128, 128], F32, tag="s",
                                      name=f"wrep{t}")
                    nc.tensor.matmul(wrep[:], esel[:], w16pT[:],
                                     start=True, stop=True)
                    w16m = attp.tile([128, 128], BF16, tag="w16m")
                    nc.vector.tensor_tensor(out=w16m[:], in0=wrep[:],
                                            in1=m4[:], op=ALU.mult)
                    nc.leave_named_scope(f"sm{t}", sm_sc[0], False)

                    # finish filler unit a now: its DVE add runs behind the
                    # softmax ops, freeing the psa bank well before the next
                    # step's filler matmuls need it
                    if units:
                        pa_compute_fin(units[0])

                    # (b) attention pooling on PE: attnT[k][c, n] =
                    #     sum_j atp_j[:, k]^T @ w16m[:, block j]
                    sc_ = nc.enter_named_scope(f"att{t}", False)
                    # chunk j holds the (n,p) rows of batches 8j..8j+8, whose
                    # weights sit at w16m cols 32j..32j+8; each (k,j) matmul
                    # fills its own 8-col strip of attnT block k
                    pool = pl_ps.tile([128, KH * 32], F32, tag="pool")
                    for k in range(KH):
                        for j in range(NPC):
                            nc.tensor.matmul(
                                pool[:, 32 * k + 8 * j:32 * k + 8 * j + 8],
                                atp[j][:, 128 * k:128 * (k + 1)],
                                w16m[:, 32 * j:32 * j + 8],
                                start=True, stop=True)
                    attn_sb = attp.tile([128, KH * 32], BF16, tag="attn_sb")
                    nc.vector.tensor_copy(attn_sb[:], pool[:])
                    attnT = [attn_sb[:, 32 * k:32 * k + 32]
                             for k in range(KH)]
                    nc.leave_named_scope(f"att{t}", sc_[0], False)

                    # per-step state: blocks 4q..4q+3 of quad q live on
                    # partitions 32g'..32g'+31 of the quad's PSUM bank
                    c_new = [blkp.tile([128, 128], F32, tag="c", bufs=4,
                                       name=f"cn{q}_{t}") for q in range(2)]
                    hbf = [blkp.tile([128, 128], BF16, tag="hbf",
                                     name=f"hbf{q}_{t}") for q in range(2)]

                    def quad_math(q, pq):
                        # all four blocks of the quad, read from PSUM
                        sio = blkp.tile([128, 384], F32, tag="sio")
                        nc.scalar.activation(sio[:], pq[:, 0:384], AF.Sigmoid)
                        tg = blkp.tile([128, 128], F32, tag="tg")
                        nc.scalar.activation(tg[:], pq[:, 384:512], AF.Tanh)
                        m1 = blkp.tile([128, 128], F32, tag="m1")
                        nc.vector.tensor_tensor(out=m1[:], in0=sio[:, 0:128],
                                                in1=tg[:], op=ALU.mult)
                        m2 = blkp.tile([128, 128], F32, tag="m2")
                        nc.vector.tensor_tensor(out=m2[:], in0=sio[:, 128:256],
                                                in1=c_b[q][:], op=ALU.mult)
                        nc.vector.tensor_add(c_new[q][:], m1[:], m2[:])
                        tcn = blkp.tile([128, 128], F32, tag="tcn")
                        nc.scalar.activation(tcn[:], c_new[q][:], AF.Tanh)
                        nc.vector.tensor_tensor(out=hbf[q][:],
                                                in0=sio[:, 256:384],
                                                in1=tcn[:], op=ALU.mult)
                        row = (q * t_steps + t) * 128
                        nc.sync.dma_start(out_d[row:row + 128, :], hbf[q][:])

                    # (f) attn-parts for quad 0 then quad0 math from PSUM
                    sc_ = nc.enter_named_scope(f"f05_{t}", False)
                    for k in range(KH, K2):
                        for gp in range(4):
                            smm(pq0, gp, attnT[k - KH], w2[k][:, gsl(0, gp)],
                                k == K2 - 1)
                    quad_math(0, pq0)
                    nc.leave_named_scope(f"f05_{t}", sc_[0], False)

                    # (h) attn-parts for quad 1 + quad1 math
                    sc_ = nc.enter_named_scope(f"h67_{t}", False)
                    for k in range(KH, K2):
                        for gp in range(4):
                            smm(pq1, gp, attnT[k - KH], w2[k][:, gsl(1, gp)],
                                k == K2 - 1)
                    quad_math(1, pq1)
                    nc.leave_named_scope(f"h67_{t}", sc_[0], False)

                    # phase-A filler inside the quad-math wait window
                    sc_ = nc.enter_named_scope(f"pa{t}", False)
                    if len(units) > 1:
                        pa_compute_mm(units[1])
                    nc.leave_named_scope(f"pa{t}", sc_[0], False)

                    # (i..l) PE transposes h->hT interleaved with next-step
                    # scores/h-parts: quad-0-derived work runs between the
                    # two transposes so the PE isn't stalled on quad1 math
                    sc_ = nc.enter_named_scope(f"nxt{t}", False)
                    if not last:
                        def score_strip(hTsl, k):
                            for j in range(NPC):
                                nc.tensor.matmul(
                                    ps_s[32 * j:32 * j + 8, :],
                                    hTsl[k][:, 8 * j:8 * j + 8],
                                    at[k][:, 128 * j:128 * (j + 1)],
                                    start=(k == 0), stop=(k == KH - 1),
                                    tile_position=(0, 32 * j),
                                    skip_group_check=True)

                        tp = pt_ps.tile([128, 128], BF16, tag="tp")
                        nc.tensor.transpose(tp[:], hbf[0][:], ident[:])
                        ht0 = htp.tile([128, 128], BF16, tag="htq", bufs=6)
                        nc.vector.tensor_copy(ht0[:], tp[:])
                        hTq_new = [ht0]
                        hT_new = None
                        ps_s = pss_p.tile([128, 128], F32, tag="s")
                        hsl0 = [ht0[:, 32 * k:32 * k + 32] for k in range(4)]
                        for k in range(4):
                            score_strip(hsl0, k)
                        pq0n = psg_p.tile([128, 512], F32, tag="g",
                                          name=f"pq0_{t}")
                        umm(pq0n, u_next, 0)
                        for k in range(4):
                            for gp in range(4):
                                smm(pq0n, gp, hsl0[k],
                                    w2[k][:, gsl(0, gp)], False)
                        # quad 1 transpose + remaining halves
                        tp1 = pt_ps.tile([128, 128], BF16, tag="tp")
                        nc.tensor.transpose(tp1[:], hbf[1][:], ident[:])
                        ht1 = htp.tile([128, 128], BF16, tag="htq", bufs=6)
                        nc.vector.tensor_copy(ht1[:], tp1[:])
                        hTq_new.append(ht1)
                        hT_new = ht_slices(hTq_new)
                        for k in range(4, KH):
                            score_strip(hT_new, k)
                        for k in range(4, KH):
                            for gp in range(4):
                                smm(pq0n, gp, hT_new[k],
                                    w2[k][:, gsl(0, gp)], False)
                        pq1n = psg_p.tile([128, 512], F32, tag="g",
                                          name=f"pq1_{t}")
                        umm(pq1n, u_next, 1)
                        for k in range(KH):
                            for gp in range(4):
                                smm(pq1n, gp, hT_new[k],
                                    w2[k][:, gsl(1, gp)], False)
                        pq0 = pq0n
                        pq1 = pq1n
                    nc.leave_named_scope(f"nxt{t}", sc_[0], False)

                    # phase-A unit-b finish + next step's fetches (late in
                    # the step so the DMA HW is quiet in the scatter window)
                    sc_ = nc.enter_named_scope(f"paf{t}", False)
                    for u in units[1:]:
                        pa_compute_fin(u)
                    for u in pa_sched(t + 1):
                        pa_fetch_unit(u)
                    nc.leave_named_scope(f"paf{t}", sc_[0], False)

                    if not last:
                        hT = hT_new
                        c_b = c_new
                        u_t = u_next

    nc.compile()
    return nc


def prepare_inputs(x, A, Wx, Wh, Wattn, b, t_steps=T):
    """Host-side sharding + layout prep. Returns list of per-core input maps."""
    x = np.asarray(x, dtype=np.float32)
    A = np.asarray(A, dtype=np.float32)
    Wx = np.asarray(Wx, dtype=np.float32)
    Wh = np.asarray(Wh, dtype=np.float32)
    Wattn = np.asarray(Wattn, dtype=np.float32)
    b = np.asarray(b, dtype=np.float32)

    perm = _gate_perm()
    wx_p = np.ascontiguousarray(Wx[:, perm]).astype(BF)
    w2_p = np.ascontiguousarray(np.vstack([Wh, Wattn])[:, perm]).astype(BF)
    b128 = np.ascontiguousarray(
        np.broadcast_to(b[perm], (128, G))).astype(BF)
    # scores diagonal-extraction mask: strip j holds batches 8j..8j+8 at
    # partitions 32j+m; batch 8j+m's own scores sit at cols m*16+p
    mask = np.zeros((128, 128), dtype=BF)
    for j in range(NPC):
        for m in range(8):
            mask[32 * j + m, m * P:(m + 1) * P] = 1
    # block-diagonal scatter mask: keeps (dn*16+p, 40j+dn) of the
    # replicated wT tile -> W16m[np', 32j + n] = w[n, p] for n = 8j+dn
    # pool-scatter mask: chunk j's (dn,p) row keeps batch 8j+dn, which the
    # Esel-replicated tile holds at col 32j+dn
    m4 = np.zeros((128, 128), dtype=BF)
    for j in range(NPC):
        for dn in range(8):
            for p in range(P):
                m4[dn * 16 + p, 32 * j + dn] = 1
    esel = np.tile(np.eye(P, dtype=BF), (1, 8))
    ident = np.eye(128, dtype=BF)
    in_maps = []
    for c in range(NCORES):
        x_c = x[c * NB:(c + 1) * NB, :t_steps]          # (NB, t, D)
        xr = x_c.transpose(1, 0, 2).reshape(t_steps * NB, D)  # t-major rows
        xT = np.ascontiguousarray(xr.T).astype(BF)       # (D, t*NB)
        A_c = A[c * NB:(c + 1) * NB].reshape(NB, H, P)
        at_c = np.ascontiguousarray(
            A_c.transpose(1, 0, 2).reshape(H, NB * P)).astype(BF)
        atp_c = np.ascontiguousarray(
            A_c.transpose(0, 2, 1).reshape(NB * P, H)).astype(BF)
        h0 = A_c.mean(axis=2).astype(np.float32)         # (NB, H)
        h0T = np.ascontiguousarray(h0.T).astype(BF)      # (H, NB)
        # quad-stacked initial cell state: block g of quad q lives on
        # partitions 32*(g%4), columns = h dims within the block
        h0q = np.empty((2 * 128, 128), dtype=np.float32)
        for g in range(8):
            q, gp = divmod(g, 4)
            h0q[q * 128 + gp * 32:q * 128 + (gp + 1) * 32, :] = \
                h0[:, g * 128:(g + 1) * 128]
        in_maps.append({
            "xT": xT, "wx": wx_p, "w2": w2_p, "b128": b128,
            "at": at_c, "atp": atp_c, "h0T": h0T, "h0q": h0q,
            "mask": mask, "m4": m4, "esel": esel, "ident": ident,
        })
    return in_maps


def kernel(x, A, Wx, Wh, Wattn, b):
    from concourse.bass_utils import run_bass_kernel_spmd

    key = T
    if key not in _NC_CACHE:
        _NC_CACHE[key] = build_nc(T)
    nc = _NC_CACHE[key]

    in_maps = prepare_inputs(x, A, Wx, Wh, Wattn, b)
    trace = bool(int(os.environ.get("KERNEL_TRACE", "0")))
    tmpdir = os.environ.get("KERNEL_TRACE_DIR") or None
    res = run_bass_kernel_spmd(nc, in_maps, core_ids=list(range(NCORES)),
                               trace=trace, tmpdir=tmpdir)
    kernel.last_result = res
    if res.exec_time_ns is not None:
        print(f"HW exec time: {res.exec_time_ns} ns")
        kernel.last_exec_time_ns = res.exec_time_ns
    # unshuffle quad-stacked bf16 output: buf[q, t, gp, n, c] ->
    # h[n, t, (4q+gp)*128 + c]
    outs = []
    for r in res.results:
        buf = np.asarray(r["out"]).reshape(2, T, 4, NB, 128)
        outs.append(buf.transpose(3, 1, 0, 2, 4).reshape(NB, T, H))
    return np.concatenate(outs, axis=0).astype(np.float32)


kernel.last_exec_time_ns = None


# revision 45
# speedup vs baseline: 2.6938x; 1.0553x over previous
"""Trainium2 Bass kernel for an attention-augmented LSTM (CaptioningRNN).

Reference computation (per batch n, T timesteps):
    A_flat = A.reshape(N, H, 16); h0 = c0 = A_flat.mean(-1)
    scores_t = (h_{t-1} @ A_flat) / sqrt(H); w = softmax(scores)
    attn_t = A_flat @ w
    a = x_t @ Wx + h_{t-1} @ Wh + attn_t @ Wattn + b
    i, f, o, g = split(a, 4); c_t = sig(f)*c + sig(i)*tanh(g); h_t = sig(o)*tanh(c_t)

Strategy: data-parallel over batch across 8 cores (32 batch rows each).
Per core:
  Phase A: U = x @ Wx + b, streamed in row-group-blocked units (Wx read 4x
           instead of 16x), staged to DRAM bf16, paced 2 units/step into the
           recurrence's idle windows.
  Phase B: 64 recurrent steps. Gate matmul = [h; attn] (2048-dim contraction,
           bf16) against W2 = [Wh; Wattn] with gate-interleaved columns so each
           512-column block yields a full 128-dim slice of (i,f,o,g) and thus a
           128-dim slice of h/c. Attention scores on the tensor engine (all
           batch pairs, diagonal via mask + strided reduce). Attention POOLING
           also on the tensor engine: softmax weights are scattered into a
           block-diagonal [128np, 4x32n] operand (stream-transpose + replicated
           DMA + static mask) and contracted against a (n,p)-major copy of A
           (atp), yielding attnT chunks directly in [h,n] layout. h transposed
           back to hT layout with PE transpose-mode matmuls (not DMA).

Weight-matrix column order (gate interleave): block j (512 cols) holds
original columns [i|f|o|g][j*128:(j+1)*128]. The same permutation is applied
to Wx, b and hence U.
"""

import math
import os

import numpy as np
import ml_dtypes

import concourse.bass as bass
import concourse.mybir as mybir
import concourse.tile as tile
from concourse import bacc

N, T, D, H = 256, 64, 1024, 1024
NCORES = 8
NB = N // NCORES          # 32 batch rows per core
G = 4 * H                 # 4096 gate columns
P = 16                    # attention positions (4x4)
KH = H // 128             # 8 contraction chunks for h
K2 = (2 * H) // 128       # 16 contraction chunks for [h; attn]
GB = G // 512             # 8 gate blocks of 512
NPC = (NB * P) // 128     # 4 chunks of the (n,p) axis
F32 = mybir.dt.float32
BF16 = mybir.dt.bfloat16
BF = ml_dtypes.bfloat16

AF = mybir.ActivationFunctionType
ALU = mybir.AluOpType
AXX = mybir.AxisListType.X

_NC_CACHE = {}

# phase A row-group blocking: 4 groups x 4 row-tiles
MG = 4                    # row-tile groups
MPG = 4                   # row-tiles per group (each row-tile = 4 timesteps)
N_ROW_TILES = MG * MPG    # 16


def _gate_perm():
    """perm[new_col] = old_col for the gate-interleaved layout."""
    perm = np.empty(G, dtype=np.int64)
    for j in range(GB):
        for s in range(4):  # i, f, o, g
            perm[j * 512 + s * 128:(j * 512 + (s + 1) * 128)] = np.arange(
                s * H + j * 128, s * H + (j + 1) * 128)
    return perm


def build_nc(t_steps=T):
    """Build the SPMD Bass program (identical on all cores)."""
    nc = bacc.Bacc("TRN2", target_bir_lowering=False, debug=False,
                   num_devices=NCORES)

    xT_d = nc.dram_tensor("xT", [D, t_steps * NB], BF16, kind="ExternalInput")
    wx_d = nc.dram_tensor("wx", [D, G], BF16, kind="ExternalInput")
    w2_d = nc.dram_tensor("w2", [2 * H, G], BF16, kind="ExternalInput")
    b128_d = nc.dram_tensor("b128", [128, G], BF16, kind="ExternalInput")
    at_d = nc.dram_tensor("at", [H, NB * P], BF16, kind="ExternalInput")
    atp_d = nc.dram_tensor("atp", [NB * P, H], BF16, kind="ExternalInput")
    h0T_d = nc.dram_tensor("h0T", [H, NB], BF16, kind="ExternalInput")
    h0q_d = nc.dram_tensor("h0q", [2 * 128, 128], F32, kind="ExternalInput")
    mask_d = nc.dram_tensor("mask", [128, 128], BF16, kind="ExternalInput")
    m4_d = nc.dram_tensor("m4", [128, 128], BF16, kind="ExternalInput")
    esel_d = nc.dram_tensor("esel", [16, 128], BF16, kind="ExternalInput")
    ident_d = nc.dram_tensor("ident", [128, 128], BF16, kind="ExternalInput")
    # quad-stacked bf16 output: row ((q*t + t)*128 + 32*gp + n), col c
    # holds h[n, t, (4q+gp)*128 + c]; host unshuffles + converts to f32
    out_d = nc.dram_tensor("out", [2 * t_steps * 128, 128], BF16,
                           kind="ExternalOutput")

    with tile.TileContext(nc) as tc:
        with tc.tile_pool(name="dram", bufs=1, space="DRAM") as dpool:
            # quad-stacked U: row (t*128 + gp*32 + n), col (q*512 + c)
            # holds U[t, n, gate block 4q+gp, c]
            u_dram = dpool.tile([t_steps * 128, 2 * 512], BF16)


            with tc.tile_pool(name="res", bufs=1) as res, \
                 tc.tile_pool(name="ht", bufs=20) as htp, \
                 tc.tile_pool(name="u", bufs=2) as up, \
                 tc.tile_pool(name="st", bufs=2) as stp, \
                 tc.tile_pool(name="att", bufs=2) as attp, \
                 tc.tile_pool(name="blk", bufs=2) as blkp, \
                 tc.tile_pool(name="pax", bufs=8) as paxp, \
                 tc.tile_pool(name="paw", bufs=2) as pawp, \
                 tc.tile_pool(name="pab", bufs=2) as pabp, \
                 tc.tile_pool(name="pau", bufs=2) as pau, \
                 tc.tile_pool(name="psg", bufs=3, space="PSUM") as psg_p, \
                 tc.tile_pool(name="psa", bufs=2, space="PSUM") as pa_ps, \
                 tc.tile_pool(name="psx", bufs=1, space="PSUM") as psx_p:
                pl_ps = psx_p
                pt_ps = psx_p
                pss_p = psx_p

                # ---------------- phase A machinery ----------------
                # unit u = (mg, g, m): row-group mg, gate block g, local
                # row-tile m (global row-tile 4*mg + m). Order: mg, g, m.
                pa_xm = {}       # (mg, m) -> xm tile
                pa_wx = {}       # (mg, g) -> (wxg tile, b-slice tile)

                TNB = t_steps * NB

                def pa_fetch_xm(mg, m):
                    if (mg, m) in pa_xm or mg >= MG:
                        return
                    xm = paxp.tile([128, KH * 128], BF16, tag="xm")
                    rt = 4 * mg + m
                    # one DMA: dst (i, d, j) <- xT[128d + i, 128 rt + j]
                    b_ = xT_d[0:128, rt * 128:(rt + 1) * 128]
                    src = bass.AP(b_.tensor, b_.offset,
                                  [b_.ap[0], [128 * TNB, KH], b_.ap[1]])
                    nc.gpsimd.dma_start(
                        xm[:].rearrange("p (d j) -> p d j", d=KH), src)
                    pa_xm[(mg, m)] = xm

                def pa_fetch_unit(u):
                    mg, g = u // (GB * MPG), (u // MPG) % GB
                    if (mg, g) in pa_wx:
                        return
                    for m in range(MPG):       # xm for this group
                        pa_fetch_xm(mg, m)
                    if g >= 4:                 # trickle next group's xm
                        pa_fetch_xm(mg + 1, g - 4)
                    wxg = pawp.tile([128, KH * 512], BF16, tag="wxg")
                    b_ = wx_d[0:128, g * 512:(g + 1) * 512]
                    src = bass.AP(b_.tensor, b_.offset,
                                  [b_.ap[0], [128 * G, KH], b_.ap[1]])
                    nc.gpsimd.dma_start(
                        wxg[:].rearrange("p (d c) -> p d c", d=KH), src)
                    bsl = pabp.tile([128, 512], BF16, tag="bsl")
                    nc.gpsimd.dma_start(bsl[:], b128_d[:, g * 512:(g + 1) * 512])
                    pa_wx[(mg, g)] = (wxg, bsl)

                pa_pend = {}

                def pa_compute_mm(u):
                    # PE part only; finish (DVE add + store) issued later
                    mg, g, m = u // (GB * MPG), (u // MPG) % GB, u % MPG
                    wxg, bsl = pa_wx[(mg, g)]
                    xm = pa_xm[(mg, m)]
                    ps = pa_ps.tile([128, 512], F32, tag="ps")
                    for d_ in range(KH):
                        nc.tensor.matmul(ps[:], xm[:, d_ * 128:(d_ + 1) * 128],
                                         wxg[:, d_ * 512:(d_ + 1) * 512],
                                         start=(d_ == 0), stop=(d_ == KH - 1))
                    pa_pend[u] = (ps, bsl)

                def pa_compute_fin(u):
                    mg, g, m = u // (GB * MPG), (u // MPG) % GB, u % MPG
                    ps, bsl = pa_pend.pop(u)
                    us = pau.tile([128, 512], BF16, tag="us")
                    nc.vector.tensor_add(us[:], ps[:], bsl[:])
                    rt = 4 * mg + m
                    q, gp = divmod(g, 4)
                    # one DMA: us row 32r + n -> u_dram[(4rt+r)*128 + 32gp + n]
                    b_ = u_dram[4 * rt * 128 + gp * 32:
                                4 * rt * 128 + gp * 32 + 32,
                                q * 512:(q + 1) * 512]
                    dst = bass.AP(b_.tensor, b_.offset,
                                  [[128 * 1024, 4], b_.ap[0], b_.ap[1]])
                    (nc.sync if u % 2 == 0 else nc.scalar).dma_start(dst,
                                                                     us[:])
                    # free consumed group resources at unit boundaries
                    if m == MPG - 1 and g == GB - 1:
                        for mm in range(MPG):
                            pa_xm.pop((mg, mm), None)
                    if m == MPG - 1:
                        pa_wx.pop((mg, g), None)

                def pa_compute_unit(u):
                    pa_compute_mm(u)
                    pa_compute_fin(u)

                N_UNITS = MG * GB * MPG          # 128
                PRO_UNITS = GB * MPG             # group 0 upfront

                def pa_sched(t):
                    # 2 filler units/step; exhausted by step 48 (group 3's
                    # rows are needed at step 48, so later pacing is unsound)
                    return range(min(PRO_UNITS + 2 * t, N_UNITS),
                                 min(PRO_UNITS + 2 * (t + 1), N_UNITS))

                # ---------------- static tiles ----------------
                # spread the 16 MB w2 load across four queues so startup
                # isn't serialized on one DMA queue
                # gpsimd is reserved for phase-A fetches at startup so the
                # prologue matmuls can begin while w2 streams on sync+scalar
                qs = [nc.sync, nc.scalar]
                w2 = []
                for k in range(K2):
                    t_ = res.tile([128, G], BF16, tag=f"w2_{k}")
                    qs[k % 2].dma_start(t_[:], w2_d[k * 128:(k + 1) * 128, :])
                    w2.append(t_)
                at_all = res.tile([128, KH * NB * P], BF16, tag="at_all")
                for k in range(KH):
                    nc.sync.dma_start(
                        at_all[:, k * NB * P:(k + 1) * NB * P],
                        at_d[k * 128:(k + 1) * 128, :])
                at = [at_all[:, k * NB * P:(k + 1) * NB * P]
                      for k in range(KH)]
                atp = []
                for j in range(NPC):
                    t_ = res.tile([128, H], BF16, tag=f"atp{j}")
                    nc.sync.dma_start(t_[:], atp_d[j * 128:(j + 1) * 128, :])
                    atp.append(t_)
                mask = res.tile([128, 128], BF16, tag="mask")
                nc.sync.dma_start(mask[:], mask_d[:])
                m4 = res.tile([128, 128], BF16, tag="m4")
                nc.sync.dma_start(m4[:], m4_d[:])
                esel = res.tile([16, 128], BF16, tag="esel")
                nc.sync.dma_start(esel[:], esel_d[:])
                ident = res.tile([128, 128], BF16, tag="ident")
                nc.sync.dma_start(ident[:], ident_d[:])

                # phase A prologue: group 0 complete (covers steps 0..15),
                # fetching one gate-block ahead of compute
                pa_fetch_unit(0)
                for u in range(PRO_UNITS):
                    pa_fetch_unit(u + MPG)
                    pa_compute_unit(u)
                for u in pa_sched(0):
                    pa_fetch_unit(u)

                def ht_slices(tq):
                    return [tq[k // 4][:, 32 * (k % 4):32 * (k % 4) + 32]
                            for k in range(KH)]

                hTq = []
                for q in range(2):
                    t_ = htp.tile([128, 128], BF16, tag="htq", bufs=6)
                    for gp in range(4):
                        k = 4 * q + gp
                        nc.sync.dma_start(t_[:, 32 * gp:32 * gp + 32],
                                          h0T_d[k * 128:(k + 1) * 128, :])
                    hTq.append(t_)
                hT = ht_slices(hTq)
                c_b = []
                for q in range(2):
                    t_ = blkp.tile([128, 128], F32, tag="c", bufs=4,
                                   name=f"c0_{q}")
                    nc.sync.dma_start(t_[:], h0q_d[q * 128:(q + 1) * 128, :])
                    c_b.append(t_)

                u_t = up.tile([128, 2 * 512], BF16, tag="u")
                nc.sync.dma_start(u_t[:], u_dram[0:128, :])

                inv_sqrt_h = 1.0 / math.sqrt(H)

                def gsl(q, gp):
                    g = 4 * q + gp
                    return slice(g * 512, (g + 1) * 512)

                def smm(pq, gp, lhs, rhs, stop):
                    # accumulate into the 32-row strip of the quad bank
                    if isinstance(lhs, tile.Tile):
                        lhs = lhs[:]
                    nc.tensor.matmul(pq[32 * gp:32 * gp + 32, :], lhs, rhs,
                                     start=False, stop=stop,
                                     tile_position=(0, 32 * gp),
                                     skip_group_check=True)

                def umm(pq, u, q):
                    # seed the whole quad bank with U via identity matmul
                    nc.tensor.matmul(pq[:], ident[:],
                                     u[:, q * 512:(q + 1) * 512],
                                     start=True, stop=False,
                                     skip_group_check=True)

                def score_mms(ps4, hTsl):
                    # scores in 4 col-tiled strips: strip j covers batches
                    # 8j..8j+8 (M=8 rows at partitions 32j..32j+8, N=128)
                    for k in range(KH):
                        for j in range(NPC):
                            nc.tensor.matmul(
                                ps4[32 * j:32 * j + 8, :],
                                hTsl[k][:, 8 * j:8 * j + 8],
                                at[k][:, 128 * j:128 * (j + 1)],
                                start=(k == 0), stop=(k == KH - 1),
                                tile_position=(0, 32 * j),
                                skip_group_check=True)

                # ---- prologue: scores S_0 + h-parts of both quads ----
                ps_s = pss_p.tile([128, 128], F32, tag="s")
                score_mms(ps_s, hT)
                pq0 = psg_p.tile([128, 512], F32, tag="g", name="pq0")
                umm(pq0, u_t, 0)
                for k in range(KH):
                    for gp in range(4):
                        smm(pq0, gp, hT[k], w2[k][:, gsl(0, gp)], False)
                pq1 = psg_p.tile([128, 512], F32, tag="g", name="pq1")
                umm(pq1, u_t, 1)
                for k in range(KH):
                    for gp in range(4):
                        smm(pq1, gp, hT[k], w2[k][:, gsl(1, gp)], False)

                for t in range(t_steps):
                    last = (t + 1 >= t_steps)
                    if not last:
                        u_next = up.tile([128, 2 * 512], BF16, tag="u")
                        nc.scalar.dma_start(
                            u_next[:], u_dram[(t + 1) * 128:(t + 2) * 128, :])

                    # PE filler for the softmax/scatter window
                    units = list(pa_sched(t))
                    if units:
                        pa_compute_mm(units[0])

                    # (a) softmax chain for step t (scores psum -> W16m).
                    # batch n = 8j+m lives at partition 32j+m throughout.
                    sm_sc = nc.enter_named_scope(f"sm{t}", False)
                    masked = stp.tile([128, 128], F32, tag="masked", bufs=1)
                    nc.vector.tensor_tensor(out=masked[:], in0=ps_s[:],
                                            in1=mask[:], op=ALU.mult)
                    sc = stp.tile([128, P], F32, tag="sc")
                    nc.vector.tensor_reduce(
                        sc[:], masked[:].rearrange("q (d p) -> q p d", p=P),
                        axis=AXX, op=ALU.add)
                    # exp(x) = s/(1-s) with s = sigmoid(x): keeps the ACT
                    # table cache at {Sigmoid, Tanh} with no per-step reloads
                    sg = stp.tile([128, P], F32, tag="sg")
                    nc.scalar.activation(sg[:], sc[:], AF.Sigmoid,
                                         scale=float(inv_sqrt_h))
                    om = stp.tile([128, P], F32, tag="om")
                    nc.scalar.activation(om[:], sc[:], AF.Sigmoid,
                                         scale=float(-inv_sqrt_h))
                    omr = stp.tile([128, P], F32, tag="omr")
                    nc.vector.reciprocal(omr[:], om[:])
                    expw = stp.tile([128, P], F32, tag="expw")
                    sume = stp.tile([128, 1], F32, tag="sume")
                    nc.vector.scalar_tensor_tensor(
                        out=expw[:], in0=sg[:], scalar=1.0, in1=omr[:],
                        op0=ALU.mult, op1=ALU.mult, accum_out=sume[:])
                    rec = stp.tile([128, 1], F32, tag="rec")
                    nc.vector.reciprocal(rec[:], sume[:])
                    w16p = stp.tile([128, P], BF16, tag="w16p")
                    nc.vector.tensor_scalar(out=w16p[:], in0=expw[:],
                                            scalar1=rec[:], scalar2=None,
                                            op0=ALU.mult)
                    # partition scatter fully on-chip: PE-transpose w16p to
                    # [16p, 128n], replicate across partitions with a single
                    # matmul against the static tiled identity (Esel^T @ wT
                    # gives out[q, col] = w[n(col), p(q)]), then the static
                    # m4 mask keeps only the block-diagonal scatter w[n,p]
                    wtp = pt_ps.tile([128, 128], BF16, tag="tp",
                                     name=f"wtp{t}")
                    nc.tensor.transpose(wtp[0:16, :], w16p[:], ident[:])
                    w16pT = attp.tile([16, 128], BF16, tag="w16pT")
                    nc.vector.tensor_copy(w16pT[:], wtp[0:16, :])
                    wrep = pss_p.tile([128, 128], F32, tag="s",
                                      name=f"wrep{t}")
                    nc.tensor.matmul(wrep[:], esel[:], w16pT[:],
                                     start=True, stop=True)
                    w16m = attp.tile([128, 128], BF16, tag="w16m")
                    nc.vector.tensor_tensor(out=w16m[:], in0=wrep[:],
                                            in1=m4[:], op=ALU.mult)
                    nc.leave_named_scope(f"sm{t}", sm_sc[0], False)

                    # finish filler unit a now: its DVE add runs behind the
                    # softmax ops, freeing the psa bank well before the next
                    # step's filler matmuls need it
                    if units:
                        pa_compute_fin(units[0])

                    # (b) attention pooling on PE: attnT[k][c, n] =
                    #     sum_j atp_j[:, k]^T @ w16m[:, block j]
                    sc_ = nc.enter_named_scope(f"att{t}", False)
                    # chunk j holds the (n,p) rows of batches 8j..8j+8, whose
                    # weights sit at w16m cols 32j..32j+8; each (k,j) matmul
                    # fills its own 8-col strip of attnT block k
                    pool = pl_ps.tile([128, KH * 32], F32, tag="pool")
                    for k in range(KH):
                        for j in range(NPC):
                            nc.tensor.matmul(
                                pool[:, 32 * k + 8 * j:32 * k + 8 * j + 8],
                                atp[j][:, 128 * k:128 * (k + 1)],
                                w16m[:, 32 * j:32 * j + 8],
                                start=True, stop=True)
                    attn_sb = attp.tile([128, KH * 32], BF16, tag="attn_sb")
                    nc.vector.tensor_copy(attn_sb[:], pool[:])
                    attnT = [attn_sb[:, 32 * k:32 * k + 32]
                             for k in range(KH)]
                    nc.leave_named_scope(f"att{t}", sc_[0], False)

                    # per-step state: blocks 4q..4q+3 of quad q live on
                    # partitions 32g'..32g'+31 of the quad's PSUM bank
                    c_new = [blkp.tile([128, 128], F32, tag="c", bufs=4,
                                       name=f"cn{q}_{t}") for q in range(2)]
                    hbf = [blkp.tile([128, 128], BF16, tag="hbf",
                                     name=f"hbf{q}_{t}") for q in range(2)]

                    # (f,h) attn-parts for both quads
                    sc_ = nc.enter_named_scope(f"f05_{t}", False)
                    for k in range(KH, K2):
                        for gp in range(4):
                            smm(pq0, gp, attnT[k - KH], w2[k][:, gsl(0, gp)],
                                k == K2 - 1)
                    nc.leave_named_scope(f"f05_{t}", sc_[0], False)
                    sc_ = nc.enter_named_scope(f"h67_{t}", False)
                    for k in range(KH, K2):
                        for gp in range(4):
                            smm(pq1, gp, attnT[k - KH], w2[k][:, gsl(1, gp)],
                                k == K2 - 1)
                    nc.leave_named_scope(f"h67_{t}", sc_[0], False)

                    # quad math with the two quads' ACT/DVE ops interleaved
                    # so quad1's sigmoid/tanh aren't queued behind quad0's
                    # c-dependent tail (hbf1 gates the next-step transpose)
                    sc_ = nc.enter_named_scope(f"qm{t}", False)
                    sio = [blkp.tile([128, 384], F32, tag="sio",
                                     name=f"sio{q}_{t}") for q in range(2)]
                    tg = [blkp.tile([128, 128], F32, tag="tg",
                                    name=f"tg{q}_{t}") for q in range(2)]
                    tcn = [blkp.tile([128, 128], F32, tag="tcn",
                                     name=f"tcn{q}_{t}") for q in range(2)]
                    pqs = [pq0, pq1]
                    for q in range(2):
                        nc.scalar.activation(sio[q][:], pqs[q][:, 0:384],
                                             AF.Sigmoid)
                        nc.scalar.activation(tg[q][:], pqs[q][:, 384:512],
                                             AF.Tanh)
                    m1 = [None, None]
                    m2 = [None, None]
                    for q in range(2):
                        m1[q] = blkp.tile([128, 128], F32, tag="m1",
                                          name=f"m1{q}_{t}")
                        nc.vector.tensor_tensor(out=m1[q][:],
                                                in0=sio[q][:, 0:128],
                                                in1=tg[q][:], op=ALU.mult)
                        m2[q] = blkp.tile([128, 128], F32, tag="m2",
                                          name=f"m2{q}_{t}")
                        nc.vector.tensor_tensor(out=m2[q][:],
                                                in0=sio[q][:, 128:256],
                                                in1=c_b[q][:], op=ALU.mult)
                        nc.vector.tensor_add(c_new[q][:], m1[q][:], m2[q][:])
                        nc.scalar.activation(tcn[q][:], c_new[q][:], AF.Tanh)
                    for q in range(2):
                        nc.vector.tensor_tensor(out=hbf[q][:],
                                                in0=sio[q][:, 256:384],
                                                in1=tcn[q][:], op=ALU.mult)
                        row = (q * t_steps + t) * 128
                        nc.sync.dma_start(out_d[row:row + 128, :], hbf[q][:])
                    nc.leave_named_scope(f"qm{t}", sc_[0], False)

                    # phase-A filler inside the quad-math wait window
                    sc_ = nc.enter_named_scope(f"pa{t}", False)
                    if len(units) > 1:
                        pa_compute_mm(units[1])
                    nc.leave_named_scope(f"pa{t}", sc_[0], False)

                    # (i..l) PE transposes h->hT interleaved with next-step
                    # scores/h-parts: quad-0-derived work runs between the
                    # two transposes so the PE isn't stalled on quad1 math
                    sc_ = nc.enter_named_scope(f"nxt{t}", False)
                    if not last:
                        def score_strip(hTsl, k):
                            for j in range(NPC):
                                nc.tensor.matmul(
                                    ps_s[32 * j:32 * j + 8, :],
                                    hTsl[k][:, 8 * j:8 * j + 8],
                                    at[k][:, 128 * j:128 * (j + 1)],
                                    start=(k == 0), stop=(k == KH - 1),
                                    tile_position=(0, 32 * j),
                                    skip_group_check=True)

                        tp = pt_ps.tile([128, 128], BF16, tag="tp")
                        nc.tensor.transpose(tp[:], hbf[0][:], ident[:])
                        ht0 = htp.tile([128, 128], BF16, tag="htq", bufs=6)
                        nc.vector.tensor_copy(ht0[:], tp[:])
                        hTq_new = [ht0]
                        hT_new = None
                        ps_s = pss_p.tile([128, 128], F32, tag="s")
                        hsl0 = [ht0[:, 32 * k:32 * k + 32] for k in range(4)]
                        for k in range(4):
                            score_strip(hsl0, k)
                        pq0n = psg_p.tile([128, 512], F32, tag="g",
                                          name=f"pq0_{t}")
                        umm(pq0n, u_next, 0)
                        for k in range(4):
                            for gp in range(4):
                                smm(pq0n, gp, hsl0[k],
                                    w2[k][:, gsl(0, gp)], False)
                        # quad 1 transpose + remaining halves
                        tp1 = pt_ps.tile([128, 128], BF16, tag="tp")
                        nc.tensor.transpose(tp1[:], hbf[1][:], ident[:])
                        ht1 = htp.tile([128, 128], BF16, tag="htq", bufs=6)
                        nc.vector.tensor_copy(ht1[:], tp1[:])
                        hTq_new.append(ht1)
                        hT_new = ht_slices(hTq_new)
                        for k in range(4, KH):
                            score_strip(hT_new, k)
                        for k in range(4, KH):
                            for gp in range(4):
                                smm(pq0n, gp, hT_new[k],
                                    w2[k][:, gsl(0, gp)], False)
                        pq1n = psg_p.tile([128, 512], F32, tag="g",
                                          name=f"pq1_{t}")
                        umm(pq1n, u_next, 1)
                        for k in range(KH):
                            for gp in range(4):
                                smm(pq1n, gp, hT_new[k],
                                    w2[k][:, gsl(1, gp)], False)
                        pq0 = pq0n
                        pq1 = pq1n
                    nc.leave_named_scope(f"nxt{t}", sc_[0], False)

                    # phase-A unit-b finish + next step's fetches (late in
                    # the step so the DMA HW is quiet in the scatter window)
                    sc_ = nc.enter_named_scope(f"paf{t}", False)
                    for u in units[1:]:
                        pa_compute_fin(u)
                    for u in pa_sched(t + 1):
                        pa_fetch_unit(u)
                    nc.leave_named_scope(f"paf{t}", sc_[0], False)

                    if not last:
                        hT = hT_new
                        c_b = c_new
                        u_t = u_next

    nc.compile()
    return nc


def prepare_inputs(x, A, Wx, Wh, Wattn, b, t_steps=T):
    """Host-side sharding + layout prep. Returns list of per-core input maps."""
    x = np.asarray(x, dtype=np.float32)
    A = np.asarray(A, dtype=np.float32)
    Wx = np.asarray(Wx, dtype=np.float32)
    Wh = np.asarray(Wh, dtype=np.float32)
    Wattn = np.asarray(Wattn, dtype=np.float32)
    b = np.asarray(b, dtype=np.float32)

    perm = _gate_perm()
    wx_p = np.ascontiguousarray(Wx[:, perm]).astype(BF)
    w2_p = np.ascontiguousarray(np.vstack([Wh, Wattn])[:, perm]).astype(BF)
    b128 = np.ascontiguousarray(
        np.broadcast_to(b[perm], (128, G))).astype(BF)
    # scores diagonal-extraction mask: strip j holds batches 8j..8j+8 at
    # partitions 32j+m; batch 8j+m's own scores sit at cols m*16+p
    mask = np.zeros((128, 128), dtype=BF)
    for j in range(NPC):
        for m in range(8):
            mask[32 * j + m, m * P:(m + 1) * P] = 1
    # block-diagonal scatter mask: keeps (dn*16+p, 40j+dn) of the
    # replicated wT tile -> W16m[np', 32j + n] = w[n, p] for n = 8j+dn
    # pool-scatter mask: chunk j's (dn,p) row keeps batch 8j+dn, which the
    # Esel-replicated tile holds at col 32j+dn
    m4 = np.zeros((128, 128), dtype=BF)
    for j in range(NPC):
        for dn in range(8):
            for p in range(P):
                m4[dn * 16 + p, 32 * j + dn] = 1
    esel = np.tile(np.eye(P, dtype=BF), (1, 8))
    ident = np.eye(128, dtype=BF)
    in_maps = []
    for c in range(NCORES):
        x_c = x[c * NB:(c + 1) * NB, :t_steps]          # (NB, t, D)
        xr = x_c.transpose(1, 0, 2).reshape(t_steps * NB, D)  # t-major rows
        xT = np.ascontiguousarray(xr.T).astype(BF)       # (D, t*NB)
        A_c = A[c * NB:(c + 1) * NB].reshape(NB, H, P)
        at_c = np.ascontiguousarray(
            A_c.transpose(1, 0, 2).reshape(H, NB * P)).astype(BF)
        atp_c = np.ascontiguousarray(
            A_c.transpose(0, 2, 1).reshape(NB * P, H)).astype(BF)
        h0 = A_c.mean(axis=2).astype(np.float32)         # (NB, H)
        h0T = np.ascontiguousarray(h0.T).astype(BF)      # (H, NB)
        # quad-stacked initial cell state: block g of quad q lives on
        # partitions 32*(g%4), columns = h dims within the block
        h0q = np.empty((2 * 128, 128), dtype=np.float32)
        for g in range(8):
            q, gp = divmod(g, 4)
            h0q[q * 128 + gp * 32:q * 128 + (gp + 1) * 32, :] = \
                h0[:, g * 128:(g + 1) * 128]
        in_maps.append({
            "xT": xT, "wx": wx_p, "w2": w2_p, "b128": b128,
            "at": at_c, "atp": atp_c, "h0T": h0T, "h0q": h0q,
            "mask": mask, "m4": m4, "esel": esel, "ident": ident,
        })
    return in_maps


def kernel(x, A, Wx, Wh, Wattn, b):
    from concourse.bass_utils import run_bass_kernel_spmd

    key = T
    if key not in _NC_CACHE:
        _NC_CACHE[key] = build_nc(T)
    nc = _NC_CACHE[key]

    in_maps = prepare_inputs(x, A, Wx, Wh, Wattn, b)
    trace = bool(int(os.environ.get("KERNEL_TRACE", "0")))
    tmpdir = os.environ.get("KERNEL_TRACE_DIR") or None
    res = run_bass_kernel_spmd(nc, in_maps, core_ids=list(range(NCORES)),
                               trace=trace, tmpdir=tmpdir)
    kernel.last_result = res
    if res.exec_time_ns is not None:
        print(f"HW exec time: {res.exec_time_ns} ns")
        kernel.last_exec_time_ns = res.exec_time_ns
    # unshuffle quad-stacked bf16 output: buf[q, t, gp, n, c] ->
    # h[n, t, (4q+gp)*128 + c]
    outs = []
    for r in res.results:
        buf = np.asarray(r["out"]).reshape(2, T, 4, NB, 128)
        outs.append(buf.transpose(3, 1, 0, 2, 4).reshape(NB, T, H))
    return np.concatenate(outs, axis=0).astype(np.float32)


kernel.last_exec_time_ns = None
